# revision 1
# baseline (speedup 1.0000x reference)
"""Trainium2 Bass kernel for nn_AttentionLayer (sparse euclidean attention).

Math (reference):
    a      = tanh(attended @ W_A_X) + b_A_X          [L, D]
    M[i,j] = sum_d W_A[d] * (a[j,d] - a[i,d])^2      (>=0, 0 on diagonal)
    energy = softmax(-M, axis=1)
    glimpsed = energy @ source
    out    = tanh(concat([glimpsed, source]) @ W_A_combine) + b_A_combine

Rewrite used here: with b = a * W_A and wsq[j] = a[j]·b[j],
    -M[i,j] = 2*a_i·b_j - wsq_i - wsq_j.
wsq_i is constant per row and cancels in the softmax, so per query q:
    E'[k,q]   = exp(2*(a_q·b_k) - wsq_k - CSHIFT)
    energy    = E' / sum_k E'
No row-max pass is needed: the true max logit sits at k=q (M=0), and with
CSHIFT=40 every quantity stays comfortably inside fp32 (wsq is ~[36, 61]
for the target distribution; safe for wsq up to ~125).

Distribution: queries (rows) sharded 8 ways; every core holds the full
key-side tensors (b^T, wsq, source) which it computes itself from the
replicated attended^T input. No collectives.

Layouts (per core, transposed so no on-chip transposes are ever needed):
    mm1:  S^T[k,q]  = bT[d,k].T @ aqT[d,q]     (PSUM [128k, 512q])
    exp:  E'^T[k,q] = Exp(2*S^T + bias[k])     (ACT, per-partition bias)
    mm2:  G^T[s,q] += srcN[k,s].T @ E'^T[k,q]  (accumulate over 64 k-tiles)
    den:  den[1,q] += ones[k,1].T @ E'^T[k,q]
    comb: out^T[s',q] = tanh(Wc[c,s'].T @ [G^T/den ; srcT_q]) + b_c
Matmul operands are bf16 (PSUM accumulation is fp32); softmax bias,
normalization and outputs are fp32.

v2 notes (from the 305us baseline trace):
- DMA emission order: query-phase + prologue inputs first, srcN last.
- Prologue work split across ACT and DVE (square on both) and wsq's
  W_A weighting folded into the reduction matmul's stationary operand.
- wsq negation folded into the PSUM->SBUF copy; per-l-tile scatter DMAs.
- 1/den broadcast via a PE rank-1 matmul (ones-row) instead of a
  DRAM round trip.
- psS bufs=3 so mm1 runs two k-tiles ahead of exp; denominator matmul
  group emission delayed so a single psD bank never stalls the PE.
"""

import numpy as np

L = 8192
D = 256
S = 256
NCORES = 8
Q = L // NCORES          # 1024 queries per core
KT = 128                 # key tile (PSUM partition dim)
NK = L // KT             # 64 key tiles
LT = 512                 # prologue l-tile width
NL = L // LT             # 16 prologue tiles
QT = 512                 # query tile (PSUM free dim)
NQ = Q // QT             # 2 query tiles per core
CSHIFT = 40.0

_cache = {}


def _build():
    import concourse.bass as bass
    import concourse.tile as tile
    from concourse import bacc, mybir

    F32 = mybir.dt.float32
    BF16 = mybir.dt.bfloat16
    AF = mybir.ActivationFunctionType
    ALU = mybir.AluOpType

    nc = bacc.Bacc("TRN2", target_bir_lowering=False, debug=False)

    attT_d = nc.dram_tensor("attT", [2, 128, L], BF16, kind="ExternalInput")
    attTq_d = nc.dram_tensor("attTq", [2, 128, Q], BF16, kind="ExternalInput")
    srcN_d = nc.dram_tensor("srcN", [L, S], BF16, kind="ExternalInput")
    srcTq_d = nc.dram_tensor("srcTq", [2, 128, Q], BF16, kind="ExternalInput")
    waxT_d = nc.dram_tensor("waxT", [2, 2, 128, 128], BF16, kind="ExternalInput")
    bax_d = nc.dram_tensor("bax", [2, 128, 1], F32, kind="ExternalInput")
    wa_d = nc.dram_tensor("wa", [2, 128, 1], F32, kind="ExternalInput")
    wabf_d = nc.dram_tensor("wabf", [2, 128, 1], BF16, kind="ExternalInput")
    wc_d = nc.dram_tensor("wc", [4, 2, 128, 128], BF16, kind="ExternalInput")
    bac_d = nc.dram_tensor("bac", [2, 128, 1], F32, kind="ExternalInput")
    out_d = nc.dram_tensor("out", [2, 128, Q], F32, kind="ExternalOutput")

    with tile.TileContext(nc) as tc:
        with tc.tile_pool(name="persist", bufs=1) as persist:
            bT = persist.tile([128, 2, L], BF16, tag="bT")
            srcN_sb = persist.tile([128, NK, S], BF16, tag="srcN")
            aq = persist.tile([128, 2, Q], BF16, tag="aq")
            srcTq_sb = persist.tile([128, 2, Q], BF16, tag="srcTq")
            attTq_sb = persist.tile([128, 2, Q], BF16, tag="attTq")
            waxT_sb = persist.tile([128, 2, 2, 128], BF16, tag="waxT")
            wc_sb = persist.tile([128, 4, 2, 128], BF16, tag="wc")
            bax_sb = persist.tile([128, 2, 1], F32, tag="bax")
            wa_sb = persist.tile([128, 2, 1], F32, tag="wa")
            wabf_sb = persist.tile([128, 2, 1], BF16, tag="wabf")
            bac_sb = persist.tile([128, 2, 1], F32, tag="bac")
            ones_sb = persist.tile([128, 1], BF16, tag="ones")
            onesrow_sb = persist.tile([1, 128], BF16, tag="onesrow")
            wsqn_t = [
                persist.tile([128, 4], F32, tag=f"wsqn{t}", name=f"wsqn{t}")
                for t in range(NL)
            ]

            # --- input DMAs, in dependency-priority order ---
            # 1) query-phase + prologue weights (small, gate everything)
            nc.sync.dma_start(out=waxT_sb[:], in_=waxT_d[:].rearrange("c m p k -> p c m k"))
            nc.sync.dma_start(out=bax_sb[:], in_=bax_d[:].rearrange("c p o -> p c o"))
            nc.sync.dma_start(out=wa_sb[:], in_=wa_d[:].rearrange("c p o -> p c o"))
            nc.sync.dma_start(out=wabf_sb[:], in_=wabf_d[:].rearrange("c p o -> p c o"))
            nc.sync.dma_start(out=attTq_sb[:], in_=attTq_d[:].rearrange("c p q -> p c q"))
            nc.vector.memset(ones_sb[:], 1.0)
            nc.vector.memset(onesrow_sb[:], 1.0)

            # 2) prologue attended^T stream (gates bT / wsq); spread the
            # dma_start triggers across engines (one queue's ~750ns/issue
            # serializes the head otherwise)
            dma_engs = [nc.sync, nc.scalar, nc.gpsimd]
            with tc.tile_pool(name="attn", bufs=6) as attn_p:
                attn_tiles = []
                for t in range(NL):
                    a_t = attn_p.tile([128, 2, LT], BF16, tag="attn")
                    for c in range(2):
                        dma_engs[(2 * t + c) % 3].dma_start(
                            out=a_t[:, c, :],
                            in_=attT_d[c, :, t * LT:(t + 1) * LT],
                        )
                    attn_tiles.append(a_t)

                # 3) main-loop / combine inputs (needed later)
                nc.sync.dma_start(out=srcTq_sb[:], in_=srcTq_d[:].rearrange("c p q -> p c q"))
                nc.sync.dma_start(out=wc_sb[:], in_=wc_d[:].rearrange("c m p k -> p c m k"))
                nc.sync.dma_start(out=bac_sb[:], in_=bac_d[:].rearrange("c p o -> p c o"))
                srcN_r = srcN_d[:].rearrange("(t p) s -> p t s", p=128)
                for i in range(16):
                    dma_engs[i % 3].dma_start(
                        out=srcN_sb[:, i * 4:(i + 1) * 4, :],
                        in_=srcN_r[:, i * 4:(i + 1) * 4, :],
                    )

                with tc.tile_pool(name="dr", bufs=1, space="DRAM") as dr:
                    wsq_dram = dr.tile([L], F32, tag="wsq_dram")

                    # ============ query transform: aq = a^T[:, own] ============
                    with tc.tile_pool(name="atq", bufs=2) as atq_p, \
                         tc.tile_pool(name="psQ", bufs=2, space="PSUM") as psQ:
                        for h in range(NQ):
                            ps = psQ.tile([128, 2, QT], F32, tag="psQ")
                            for m in range(2):
                                for c in range(2):
                                    nc.tensor.matmul(
                                        ps[:, m, :],
                                        waxT_sb[:, c, m, :],
                                        attTq_sb[:, c, h * QT:(h + 1) * QT],
                                        start=(c == 0), stop=(c == 1),
                                    )
                            for m in range(2):
                                at_q = atq_p.tile([128, QT], F32, tag="atq")
                                nc.scalar.activation(
                                    out=at_q[:], in_=ps[:, m, :], func=AF.Tanh
                                )
                                nc.vector.tensor_scalar_add(
                                    aq[:, m, h * QT:(h + 1) * QT], at_q[:],
                                    bax_sb[:, m, 0:1],
                                )

                    # ========== prologue: a^T -> bT, wsq (ACT/DVE split) ==========
                    with tc.tile_pool(name="at", bufs=3) as at_p, \
                         tc.tile_pool(name="sq", bufs=3) as sq_p, \
                         tc.tile_pool(name="wstage", bufs=2) as wstage_p, \
                         tc.tile_pool(name="psA", bufs=3, space="PSUM") as psA, \
                         tc.tile_pool(name="psW", bufs=2, space="PSUM") as psW:

                        def emit_mma(t):
                            ps = psA.tile([128, 2, LT], F32, tag="psA")
                            for m in range(2):
                                for c in range(2):
                                    nc.tensor.matmul(
                                        ps[:, m, :],
                                        waxT_sb[:, c, m, :],
                                        attn_tiles[t][:, c, :],
                                        start=(c == 0), stop=(c == 1),
                                    )
                            return ps

                        ps_prev = emit_mma(0)
                        for t in range(NL):
                            ps_next = emit_mma(t + 1) if t + 1 < NL else None
                            # tanh for both chunks in one ACT call (no bias)
                            at_t = at_p.tile([128, 2, LT], F32, tag="at")
                            nc.scalar.activation(
                                out=at_t[:], in_=ps_prev[:], func=AF.Tanh,
                            )
                            sq_t = sq_p.tile([128, 2, LT], BF16, tag="sq")
                            # chunk 0: bT on DVE, square on ACT
                            nc.vector.tensor_scalar(
                                bT[:, 0, t * LT:(t + 1) * LT], at_t[:, 0, :],
                                bax_sb[:, 0, 0:1], wa_sb[:, 0, 0:1],
                                op0=ALU.add, op1=ALU.mult,
                            )
                            nc.scalar.activation(
                                out=sq_t[:, 0, :], in_=at_t[:, 0, :],
                                func=AF.Square, bias=bax_sb[:, 0, 0:1], scale=1.0,
                            )
                            # chunk 1: adds/muls on DVE, square on ACT
                            at1 = at_p.tile([128, LT], F32, tag="at1")
                            nc.vector.tensor_scalar_add(
                                at1[:], at_t[:, 1, :], bax_sb[:, 1, 0:1]
                            )
                            nc.vector.tensor_scalar_mul(
                                bT[:, 1, t * LT:(t + 1) * LT], at1[:],
                                wa_sb[:, 1, 0:1],
                            )
                            nc.scalar.activation(
                                out=sq_t[:, 1, :], in_=at1[:], func=AF.Square,
                            )
                            # wsq = sum_d W_A[d] * (a+b)^2 : fold W_A into lhsT
                            ps_w = psW.tile([1, LT], F32, tag="psW")
                            for c in range(2):
                                nc.tensor.matmul(
                                    ps_w[:], wabf_sb[:, c, :], sq_t[:, c, :],
                                    start=(c == 0), stop=(c == 1),
                                )
                            # negate+shift while copying out of PSUM
                            wst = wstage_p.tile([1, LT], F32, tag="wst")
                            nc.vector.tensor_scalar(
                                wst[:], ps_w[:], -1.0, -CSHIFT,
                                op0=ALU.mult, op1=ALU.add,
                            )
                            nc.gpsimd.dma_start(
                                out=wsq_dram[t * LT:(t + 1) * LT], in_=wst[0:1, :]
                            )
                            # per-l-tile scatter into a dedicated [128, 4] tile
                            nc.gpsimd.dma_start(
                                out=wsqn_t[t][:],
                                in_=bass.AP(
                                    tensor=wsq_dram.tensor,
                                    offset=wsq_dram.offset + t * LT,
                                    ap=[[1, 128], [128, 4]],
                                ),
                            )
                            ps_prev = ps_next

                    # ===================== main attention loop =====================
                    with tc.tile_pool(name="eT", bufs=9) as eT_p, \
                         tc.tile_pool(name="gN", bufs=2) as gN_p, \
                         tc.tile_pool(name="ct", bufs=2) as ct_p, \
                         tc.tile_pool(name="rcp", bufs=2) as rcp_p, \
                         tc.tile_pool(name="bcast", bufs=2) as bcast_p, \
                         tc.tile_pool(name="psS", bufs=3, space="PSUM") as psS, \
                         tc.tile_pool(name="psG", bufs=2, space="PSUM") as psG, \
                         tc.tile_pool(name="psD", bufs=1, space="PSUM") as psD:

                        DELAY = 5     # k-tiles mm1/exp run ahead of mm2
                        ONES_LAG = 2  # extra lag of the denominator group
                        TAIL_AT = 4   # steady index where prev-qt tail is emitted

                        def emit_qtile(h, emit_tail_prev):
                            aq0 = aq[:, 0, h * QT:(h + 1) * QT]
                            aq1 = aq[:, 1, h * QT:(h + 1) * QT]
                            ps_g = psG.tile([128, 2, QT], F32, tag="psG")
                            ps_d = psD.tile([1, QT], F32, tag="psD")

                            def emit_mm1(t):
                                ps_s = psS.tile([128, QT], F32, tag="s")
                                nc.tensor.matmul(
                                    ps_s[:], bT[:, 0, t * KT:(t + 1) * KT], aq0,
                                    start=True, stop=False,
                                )
                                nc.tensor.matmul(
                                    ps_s[:], bT[:, 1, t * KT:(t + 1) * KT], aq1,
                                    start=False, stop=True,
                                )
                                return ps_s

                            def emit_exp(t, ps_s):
                                e_t = eT_p.tile([128, QT], BF16, tag="eT")
                                nc.scalar.activation(
                                    out=e_t[:], in_=ps_s[:], func=AF.Exp,
                                    bias=wsqn_t[t // 4][:, t % 4:t % 4 + 1],
                                    scale=2.0,
                                )
                                return e_t

                            def emit_mm2(t, e_t):
                                for m in range(2):
                                    nc.tensor.matmul(
                                        ps_g[:, m, :],
                                        srcN_sb[:, t, m * 128:(m + 1) * 128],
                                        e_t[:],
                                        start=(t == 0), stop=(t == NK - 1),
                                    )

                            def emit_ones(t, e_t):
                                nc.tensor.matmul(
                                    ps_d[:], ones_sb[:], e_t[:],
                                    start=(t == 0), stop=(t == NK - 1),
                                )

                            # prologue: run mm1/exp DELAY tiles ahead of mm2
                            ss = [emit_mm1(0)]
                            es = []
                            for t in range(DELAY):
                                es.append(emit_exp(t, ss[t]))
                                ss.append(emit_mm1(t + 1))
                            for t in range(NK):
                                if t + DELAY < NK:
                                    es.append(emit_exp(t + DELAY, ss[t + DELAY]))
                                    ss.append(emit_mm1(t + DELAY + 1) if t + DELAY + 1 < NK else None)
                                if t == TAIL_AT and emit_tail_prev is not None:
                                    emit_tail_prev()
                                emit_mm2(t, es[t])
                                if t >= ONES_LAG:
                                    emit_ones(t - ONES_LAG, es[t - ONES_LAG])
                            for t in range(NK - ONES_LAG, NK):
                                emit_ones(t, es[t])
                            return ps_g, ps_d

                        def make_tail(h, ps_g, ps_d):
                            def tail():
                                # 1/den, broadcast via rank-1 PE matmul
                                rcp = rcp_p.tile([1, QT], F32, tag="rcp")
                                nc.vector.reciprocal(out=rcp[:], in_=ps_d[:])
                                rcp_bf = rcp_p.tile([1, QT], BF16, tag="rcpbf")
                                nc.vector.tensor_copy(out=rcp_bf[:], in_=rcp[:])
                                ps_b = psS.tile([128, QT], F32, tag="s")
                                nc.tensor.matmul(
                                    ps_b[:], onesrow_sb[:], rcp_bf[:],
                                    start=True, stop=True,
                                )
                                bc = bcast_p.tile([128, QT], F32, tag="bcast")
                                nc.scalar.copy(out=bc[:], in_=ps_b[:])
                                g_n = gN_p.tile([128, 2, QT], BF16, tag="gN")
                                for m in range(2):
                                    nc.vector.tensor_tensor(
                                        out=g_n[:, m, :], in0=ps_g[:, m, :],
                                        in1=bc[:], op=ALU.mult,
                                    )
                                for m in range(2):
                                    ps_c = psS.tile([128, QT], F32, tag="s")
                                    rhss = [
                                        g_n[:, 0, :], g_n[:, 1, :],
                                        srcTq_sb[:, 0, h * QT:(h + 1) * QT],
                                        srcTq_sb[:, 1, h * QT:(h + 1) * QT],
                                    ]
                                    for c in range(4):
                                        nc.tensor.matmul(
                                            ps_c[:], wc_sb[:, c, m, :], rhss[c],
                                            start=(c == 0), stop=(c == 3),
                                        )
                                    c_t = ct_p.tile([128, QT], F32, tag="ct")
                                    nc.scalar.activation(
                                        out=c_t[:], in_=ps_c[:], func=AF.Tanh
                                    )
                                    nc.vector.tensor_scalar_add(
                                        c_t[:], c_t[:], bac_sb[:, m, 0:1]
                                    )
                                    for z in range(2):
                                        dma_engs[z % 2].dma_start(
                                            out=out_d[m, :, h * QT + z * 256:
                                                      h * QT + (z + 1) * 256],
                                            in_=c_t[:, z * 256:(z + 1) * 256],
                                        )
                            return tail

                        tail_prev = None
                        for h in range(NQ):
                            ps_g, ps_d = emit_qtile(h, tail_prev)
                            tail_prev = make_tail(h, ps_g, ps_d)
                        tail_prev()

    nc.compile()
    return nc


def _get_nc():
    if "nc" not in _cache:
        _cache["nc"] = _build()
    return _cache["nc"]


def _bf16(x):
    import ml_dtypes

    return np.ascontiguousarray(x, dtype=ml_dtypes.bfloat16)


def _prep_inputs(attended, source, W_A_X, b_A_X, W_A, W_A_combine, b_A_combine):
    f = np.float32
    att = np.asarray(attended, dtype=f)
    src = np.asarray(source, dtype=f)
    attT = _bf16(att.T.reshape(2, 128, L))
    srcN = _bf16(src)
    waxT = _bf16(
        np.asarray(W_A_X, dtype=f).reshape(2, 128, 2, 128).transpose(0, 2, 1, 3)
    )
    wc = _bf16(
        np.asarray(W_A_combine, dtype=f).reshape(4, 128, 2, 128).transpose(0, 2, 1, 3)
    )
    bax = np.ascontiguousarray(np.asarray(b_A_X, dtype=f).reshape(2, 128, 1))
    wa = np.ascontiguousarray(np.asarray(W_A, dtype=f).reshape(2, 128, 1))
    wabf = _bf16(wa)
    bac = np.ascontiguousarray(np.asarray(b_A_combine, dtype=f).reshape(2, 128, 1))

    in_maps = []
    for i in range(NCORES):
        sl = slice(i * Q, (i + 1) * Q)
        attTq = _bf16(att[sl].T.reshape(2, 128, Q))
        srcTq = _bf16(src[sl].T.reshape(2, 128, Q))
        in_maps.append({
            "attT": attT, "attTq": attTq, "srcN": srcN, "srcTq": srcTq,
            "waxT": waxT, "bax": bax, "wa": wa, "wabf": wabf,
            "wc": wc, "bac": bac,
        })
    return in_maps


def _run(in_maps, trace=False):
    from concourse.bass_utils import run_bass_kernel_spmd

    nc = _get_nc()
    res = run_bass_kernel_spmd(nc, in_maps, list(range(NCORES)), trace=trace)
    _cache["last_result"] = res
    out = np.empty((L, S), dtype=np.float32)
    for i in range(NCORES):
        o = res.results[i]["out"]          # [2, 128, Q] = combined^T chunks
        out[i * Q:(i + 1) * Q, :] = np.asarray(o, dtype=np.float32).reshape(S, Q).T
    return out


def kernel(**inputs):
    in_maps = _prep_inputs(**inputs)
    return _run(in_maps, trace=False)



# revision 4
# speedup vs baseline: 1.0192x; 1.0192x over previous
"""Trainium2 Bass kernel for nn_AttentionLayer (sparse euclidean attention).

Math (reference):
    a      = tanh(attended @ W_A_X) + b_A_X          [L, D]
    M[i,j] = sum_d W_A[d] * (a[j,d] - a[i,d])^2      (>=0, 0 on diagonal)
    energy = softmax(-M, axis=1)
    glimpsed = energy @ source
    out    = tanh(concat([glimpsed, source]) @ W_A_combine) + b_A_combine

Rewrite used here: with b = a * W_A and wsq[j] = a[j]·b[j],
    -M[i,j] = 2*a_i·b_j - wsq_i - wsq_j.
wsq_i is constant per row and cancels in the softmax, so per query q:
    E'[k,q]   = exp(2*(a_q·b_k) - wsq_k - CSHIFT)
    energy    = E' / sum_k E'
No row-max pass is needed: the true max logit sits at k=q (M=0), and with
CSHIFT=40 every quantity stays comfortably inside fp32 (wsq is ~[36, 61]
for the target distribution; safe for wsq up to ~125).

Distribution: queries (rows) sharded 8 ways; every core holds the full
key-side tensors (b^T, wsq, source) which it computes itself from the
replicated attended^T input. No collectives.

Layouts (per core, transposed so no on-chip transposes are ever needed):
    mm1:  S^T[k,q]  = bT[d,k].T @ aqT[d,q]     (PSUM [128k, 512q])
    exp:  E'^T[k,q] = Exp(2*S^T + bias[k])     (ACT, per-partition bias)
    mm2:  G^T[s,q] += srcN[k,s].T @ E'^T[k,q]  (accumulate over 64 k-tiles)
    den:  den_acc[128,q] += E'^T  on GPSIMD; one ones-matmul at the end
          replicates sum_k over all partitions.
    comb: out^T[s',q] = tanh(Wc[c,s'].T @ [G^T/den ; srcT_q]) + b_c

v3 changes (from the 260us v2 trace):
- den removed from PE (was 64x 307ns [128,1,512] matmuls per q-tile):
  GPSIMD accumulates E' tiles, one [128,128]x[128,512] ones-matmul
  broadcasts the partition-sum, reciprocal_approx_fast on 128 lanes
  (the old [1,512] DVE reciprocal alone was 3.3us).
- wsq transpose ([1,L] row -> [128,64] per-k-tile bias) via a single
  DRAM round trip with 512B-contiguous descriptors + one PE transpose,
  instead of 16K 4-byte scatter descriptors that poisoned every DMA
  queue for the whole prologue.
- All DRAM inputs host-prepped partition-major so every load is a few
  big per-partition-contiguous DMAs (4-16KB lines) split between the
  two HWDGE rings (sync + scalar) in dependency-priority order.
- Prologue elementwise work split DVE/ACT; squares via DVE/gpsimd mults
  instead of ACT Square to keep ACT off the critical path.
"""

import numpy as np

L = 8192
D = 256
S = 256
NCORES = 8
Q = L // NCORES          # 1024 queries per core
KT = 128                 # key tile (PSUM partition dim)
NK = L // KT             # 64 key tiles
LT = 512                 # prologue l-tile width
NL = L // LT             # 16 prologue tiles
QT = 512                 # query tile (PSUM free dim)
NQ = Q // QT             # 2 query tiles per core
CSHIFT = 40.0

_cache = {}


def _build():
    import concourse.bass as bass
    import concourse.tile as tile
    from concourse import bacc, mybir, masks

    F32 = mybir.dt.float32
    BF16 = mybir.dt.bfloat16
    AF = mybir.ActivationFunctionType
    ALU = mybir.AluOpType

    nc = bacc.Bacc("TRN2", target_bir_lowering=False, debug=False)

    # host-prepped, partition-major DRAM inputs
    attT_d = nc.dram_tensor("attT", [4, 128, 2, 2048], BF16, kind="ExternalInput")
    attTq_d = nc.dram_tensor("attTq", [128, 2, Q], BF16, kind="ExternalInput")
    srcN_d = nc.dram_tensor("srcN", [2, 128, 32, S], BF16, kind="ExternalInput")
    srcTq_d = nc.dram_tensor("srcTq", [128, 2, Q], BF16, kind="ExternalInput")
    waxT_d = nc.dram_tensor("waxT", [128, 2, 2, 128], BF16, kind="ExternalInput")
    wc_d = nc.dram_tensor("wc", [128, 4, 2, 128], BF16, kind="ExternalInput")
    smalls_d = nc.dram_tensor("smalls", [128, 6], F32, kind="ExternalInput")
    nwabf_d = nc.dram_tensor("nwabf", [128, 2], BF16, kind="ExternalInput")
    out_d = nc.dram_tensor("out", [2, 128, Q], F32, kind="ExternalOutput")

    with tile.TileContext(nc) as tc:
        with tc.tile_pool(name="persist", bufs=1) as persist:
            bT = persist.tile([128, 2, L], BF16, tag="bT")
            attT_sb = persist.tile([128, 2, L], BF16, tag="attT")
            srcN_sb = persist.tile([128, NK, S], BF16, tag="srcN")
            aq = persist.tile([128, 2, Q], BF16, tag="aq")
            srcTq_sb = persist.tile([128, 2, Q], BF16, tag="srcTq")
            attTq_sb = persist.tile([128, 2, Q], BF16, tag="attTq")
            waxT_sb = persist.tile([128, 2, 2, 128], BF16, tag="waxT")
            wc_sb = persist.tile([128, 4, 2, 128], BF16, tag="wc")
            smalls_sb = persist.tile([128, 6], F32, tag="smalls")
            nwabf_sb = persist.tile([128, 2], BF16, tag="nwabf")
            ones_sb = persist.tile([128, 128], BF16, tag="ones")
            ident_sb = persist.tile([32, 32], F32, tag="ident")
            negrow = persist.tile([1, L], F32, tag="negrow")
            wsq64 = [
                persist.tile([32, 128], F32, tag=f"wsq64_{hh}", name=f"wsq64_{hh}")
                for hh in range(2)
            ]
            wsqT = persist.tile([128, NK], F32, tag="wsqT")

            bax = [smalls_sb[:, c:c + 1] for c in range(2)]
            wa = [smalls_sb[:, 2 + c:3 + c] for c in range(2)]
            bac = [smalls_sb[:, 4 + c:5 + c] for c in range(2)]

            # --- input DMAs: sync ring gets the attended stream, scalar
            # ring the source stream; FIFO order per ring = priority ---
            nc.sync.dma_start(out=waxT_sb[:], in_=waxT_d[:])
            nc.sync.dma_start(out=attTq_sb[:], in_=attTq_d[:])
            nc.scalar.dma_start(out=smalls_sb[:], in_=smalls_d[:])
            nc.scalar.dma_start(out=nwabf_sb[:], in_=nwabf_d[:])
            for t in range(4):
                nc.sync.dma_start(
                    out=attT_sb[:, :, t * 2048:(t + 1) * 2048],
                    in_=attT_d[t],
                )
            srcN_v = srcN_sb[:].rearrange("p (h t) s -> p h t s", h=2)
            for hh in range(2):
                nc.scalar.dma_start(out=srcN_v[:, hh, :, :], in_=srcN_d[hh])
            nc.scalar.dma_start(out=srcTq_sb[:], in_=srcTq_d[:])
            nc.scalar.dma_start(out=wc_sb[:], in_=wc_d[:])

            nc.vector.memset(ones_sb[:], 1.0)
            masks.make_identity(nc, ident_sb[:])

            with tc.tile_pool(name="dr", bufs=1, space="DRAM") as dr:
                wsq_dram = dr.tile([L], F32, tag="wsq_dram")

                # ============ query transform: aq = a^T[:, own] ============
                with tc.tile_pool(name="atq", bufs=2) as atq_p, \
                     tc.tile_pool(name="psQ", bufs=2, space="PSUM") as psQ:
                    for h in range(NQ):
                        ps = psQ.tile([128, 2, QT], F32, tag="psQ")
                        for m in range(2):
                            for c in range(2):
                                nc.tensor.matmul(
                                    ps[:, m, :],
                                    waxT_sb[:, c, m, :],
                                    attTq_sb[:, c, h * QT:(h + 1) * QT],
                                    start=(c == 0), stop=(c == 1),
                                )
                        for m in range(2):
                            at_q = atq_p.tile([128, QT], F32, tag="atq")
                            nc.scalar.activation(
                                out=at_q[:], in_=ps[:, m, :], func=AF.Tanh
                            )
                            nc.vector.tensor_scalar_add(
                                aq[:, m, h * QT:(h + 1) * QT], at_q[:],
                                bax[m],
                            )

                # ========== prologue: a^T -> bT, wsq ==========
                # per l-tile: mma -> tanh(ACT) -> at01=at+bax (DVE) ->
                # bT=at01*wa (DVE) -> sq=at01*at01 (DVE/gpsimd) ->
                # wsq matmuls with -W_A folded into lhsT -> psW -> negrow.
                with tc.tile_pool(name="at", bufs=3) as at_p, \
                     tc.tile_pool(name="sq", bufs=3) as sq_p, \
                     tc.tile_pool(name="psA", bufs=2, space="PSUM") as psA, \
                     tc.tile_pool(name="psW", bufs=2, space="PSUM") as psW, \
                     tc.tile_pool(name="psT", bufs=2, space="PSUM") as psT_p:

                    def emit_mma(t):
                        ps = psA.tile([128, 2, LT], F32, tag="psA")
                        for m in range(2):
                            for c in range(2):
                                nc.tensor.matmul(
                                    ps[:, m, :],
                                    waxT_sb[:, c, m, :],
                                    attT_sb[:, c, t * LT:(t + 1) * LT],
                                    start=(c == 0), stop=(c == 1),
                                )
                        return ps

                    def finish_wsq_half(hh):
                        # negrow holds -wsq; round-trip through DRAM to get
                        # it partition-distributed, then PE-transpose the
                        # [32,128] block into bias layout [128, 32].
                        nc.gpsimd.dma_start(
                            out=wsq_dram[hh * 4096:(hh + 1) * 4096],
                            in_=negrow[0:1, hh * 4096:(hh + 1) * 4096],
                        )
                        nc.gpsimd.dma_start(
                            out=wsq64[hh][:],
                            in_=bass.AP(
                                tensor=wsq_dram.tensor,
                                offset=wsq_dram.offset + hh * 4096,
                                ap=[[128, 32], [1, 128]],
                            ),
                        )
                        ps_t = psT_p.tile([128, 32], F32, tag="psT")
                        nc.tensor.transpose(
                            ps_t[:], wsq64[hh][:], ident_sb[:],
                        )
                        nc.vector.tensor_scalar_add(
                            wsqT[:, hh * 32:(hh + 1) * 32], ps_t[:], -CSHIFT
                        )

                    ps_prev = emit_mma(0)
                    for t in range(NL):
                        ps_next = emit_mma(t + 1) if t + 1 < NL else None
                        at_t = at_p.tile([128, 2, LT], F32, tag="at")
                        nc.scalar.activation(
                            out=at_t[:], in_=ps_prev[:], func=AF.Tanh,
                        )
                        sq_t = sq_p.tile([128, 2, LT], BF16, tag="sq")
                        at01 = at_p.tile([128, 2, LT], F32, tag="at01")
                        for c in range(2):
                            nc.vector.tensor_scalar_add(
                                at01[:, c, :], at_t[:, c, :], bax[c]
                            )
                            nc.vector.tensor_scalar_mul(
                                bT[:, c, t * LT:(t + 1) * LT], at01[:, c, :],
                                wa[c],
                            )
                            # square on gpsimd to keep DVE/ACT free
                            nc.gpsimd.tensor_tensor(
                                out=sq_t[:, c, :], in0=at01[:, c, :],
                                in1=at01[:, c, :], op=ALU.mult,
                            )
                        ps_w = psW.tile([1, LT], F32, tag="psW")
                        for c in range(2):
                            nc.tensor.matmul(
                                ps_w[:], nwabf_sb[:, c:c + 1], sq_t[:, c, :],
                                start=(c == 0), stop=(c == 1),
                            )
                        nc.vector.tensor_copy(
                            out=negrow[0:1, t * LT:(t + 1) * LT], in_=ps_w[:]
                        )
                        if t == NL // 2 - 1:
                            finish_wsq_half(0)
                        elif t == NL - 1:
                            finish_wsq_half(1)
                        ps_prev = ps_next

                # ===================== main attention loop =====================
                with tc.tile_pool(name="eT", bufs=9) as eT_p, \
                     tc.tile_pool(name="gN", bufs=2) as gN_p, \
                     tc.tile_pool(name="ct", bufs=2) as ct_p, \
                     tc.tile_pool(name="rcp", bufs=2) as rcp_p, \
                     tc.tile_pool(name="dacc", bufs=2) as dacc_p, \
                     tc.tile_pool(name="psS", bufs=3, space="PSUM") as psS, \
                     tc.tile_pool(name="psG", bufs=2, space="PSUM") as psG, \
                     tc.tile_pool(name="psD", bufs=1, space="PSUM") as psD:

                    DELAY = 5     # k-tiles mm1/exp run ahead of mm2
                    TAIL_AT = 4   # steady index where prev-qt tail is emitted

                    def emit_qtile(h, emit_tail_prev):
                        aq0 = aq[:, 0, h * QT:(h + 1) * QT]
                        aq1 = aq[:, 1, h * QT:(h + 1) * QT]
                        ps_g = psG.tile([128, 2, QT], F32, tag="psG")
                        den = dacc_p.tile([128, QT], F32, tag="dacc")

                        def emit_mm1(t):
                            ps_s = psS.tile([128, QT], F32, tag="s")
                            nc.tensor.matmul(
                                ps_s[:], bT[:, 0, t * KT:(t + 1) * KT], aq0,
                                start=True, stop=False,
                            )
                            nc.tensor.matmul(
                                ps_s[:], bT[:, 1, t * KT:(t + 1) * KT], aq1,
                                start=False, stop=True,
                            )
                            return ps_s

                        def emit_exp(t, ps_s):
                            e_t = eT_p.tile([128, QT], BF16, tag="eT")
                            nc.scalar.activation(
                                out=e_t[:], in_=ps_s[:], func=AF.Exp,
                                bias=wsqT[:, t:t + 1],
                                scale=2.0,
                            )
                            return e_t

                        def emit_mm2(t, e_t):
                            for m in range(2):
                                nc.tensor.matmul(
                                    ps_g[:, m, :],
                                    srcN_sb[:, t, m * 128:(m + 1) * 128],
                                    e_t[:],
                                    start=(t == 0), stop=(t == NK - 1),
                                )

                        def emit_den(t, es):
                            # gpsimd running sum of E' tiles (partition-wise)
                            if t == 0:
                                return
                            if t == 1:
                                nc.gpsimd.tensor_tensor(
                                    out=den[:], in0=es[0][:], in1=es[1][:],
                                    op=ALU.add,
                                )
                            else:
                                nc.gpsimd.tensor_tensor(
                                    out=den[:], in0=den[:], in1=es[t][:],
                                    op=ALU.add,
                                )

                        ss = [emit_mm1(0)]
                        es = []
                        for t in range(DELAY):
                            es.append(emit_exp(t, ss[t]))
                            ss.append(emit_mm1(t + 1))
                        for t in range(NK):
                            if t + DELAY < NK:
                                es.append(emit_exp(t + DELAY, ss[t + DELAY]))
                                ss.append(emit_mm1(t + DELAY + 1) if t + DELAY + 1 < NK else None)
                            if t == TAIL_AT and emit_tail_prev is not None:
                                emit_tail_prev()
                            emit_mm2(t, es[t])
                            emit_den(t, es)
                        return ps_g, den

                    def make_tail(h, ps_g, den):
                        def tail():
                            # replicate sum over partitions via ones-matmul,
                            # then fast reciprocal on all 128 lanes
                            den_bf = rcp_p.tile([128, QT], BF16, tag="denbf")
                            nc.vector.tensor_copy(out=den_bf[:], in_=den[:])
                            ps_d = psD.tile([128, QT], F32, tag="psD")
                            nc.tensor.matmul(
                                ps_d[:], ones_sb[:], den_bf[:],
                                start=True, stop=True,
                            )
                            rcp = rcp_p.tile([128, QT], F32, tag="rcp")
                            nc.vector.reciprocal_approx_fast(
                                out=rcp[:], in_=ps_d[:]
                            )
                            g_n = gN_p.tile([128, 2, QT], BF16, tag="gN")
                            for m in range(2):
                                nc.vector.tensor_tensor(
                                    out=g_n[:, m, :], in0=ps_g[:, m, :],
                                    in1=rcp[:], op=ALU.mult,
                                )
                            for m in range(2):
                                ps_c = psS.tile([128, QT], F32, tag="s")
                                rhss = [
                                    srcTq_sb[:, 0, h * QT:(h + 1) * QT],
                                    srcTq_sb[:, 1, h * QT:(h + 1) * QT],
                                    g_n[:, 0, :], g_n[:, 1, :],
                                ]
                                cidx = [2, 3, 0, 1]
                                for j in range(4):
                                    nc.tensor.matmul(
                                        ps_c[:], wc_sb[:, cidx[j], m, :], rhss[j],
                                        start=(j == 0), stop=(j == 3),
                                    )
                                c_t = ct_p.tile([128, QT], F32, tag="ct")
                                nc.scalar.activation(
                                    out=c_t[:], in_=ps_c[:], func=AF.Tanh
                                )
                                nc.vector.tensor_scalar_add(
                                    c_t[:], c_t[:], bac[m]
                                )
                                nc.sync.dma_start(
                                    out=out_d[m, :, h * QT:(h + 1) * QT],
                                    in_=c_t[:],
                                )
                        return tail

                    tail_prev = None
                    for h in range(NQ):
                        ps_g, den = emit_qtile(h, tail_prev)
                        tail_prev = make_tail(h, ps_g, den)
                    tail_prev()

    nc.compile()
    return nc


def _get_nc():
    if "nc" not in _cache:
        _cache["nc"] = _build()
    return _cache["nc"]


def _bf16(x):
    import ml_dtypes

    return np.ascontiguousarray(x, dtype=ml_dtypes.bfloat16)


def _prep_inputs(attended, source, W_A_X, b_A_X, W_A, W_A_combine, b_A_combine):
    f = np.float32
    att = np.asarray(attended, dtype=f)
    src = np.asarray(source, dtype=f)
    attT = _bf16(att.T.reshape(2, 128, 4, 2048).transpose(2, 1, 0, 3))
    srcN = _bf16(src.reshape(2, 32, 128, S).transpose(0, 2, 1, 3))
    waxT = _bf16(
        np.asarray(W_A_X, dtype=f).reshape(2, 128, 2, 128).transpose(1, 0, 2, 3)
    )
    wc = _bf16(
        np.asarray(W_A_combine, dtype=f).reshape(4, 128, 2, 128).transpose(1, 0, 2, 3)
    )
    bax = np.asarray(b_A_X, dtype=f)
    wa = np.asarray(W_A, dtype=f)
    bac = np.asarray(b_A_combine, dtype=f)
    smalls = np.ascontiguousarray(
        np.stack(
            [bax[:128], bax[128:], wa[:128], wa[128:], bac[:128], bac[128:]],
            axis=1,
        )
    )
    nwabf = _bf16((-wa).reshape(2, 128).T)

    in_maps = []
    for i in range(NCORES):
        sl = slice(i * Q, (i + 1) * Q)
        attTq = _bf16(att[sl].T.reshape(2, 128, Q).transpose(1, 0, 2))
        srcTq = _bf16(src[sl].T.reshape(2, 128, Q).transpose(1, 0, 2))
        in_maps.append({
            "attT": attT, "attTq": attTq, "srcN": srcN, "srcTq": srcTq,
            "waxT": waxT, "wc": wc, "smalls": smalls, "nwabf": nwabf,
        })
    return in_maps


def _run(in_maps, trace=False):
    from concourse.bass_utils import run_bass_kernel_spmd

    nc = _get_nc()
    res = run_bass_kernel_spmd(nc, in_maps, list(range(NCORES)), trace=trace)
    _cache["last_result"] = res
    out = np.empty((L, S), dtype=np.float32)
    for i in range(NCORES):
        o = res.results[i]["out"]          # [2, 128, Q] = combined^T chunks
        out[i * Q:(i + 1) * Q, :] = np.asarray(o, dtype=np.float32).reshape(S, Q).T
    return out


def kernel(**inputs):
    in_maps = _prep_inputs(**inputs)
    return _run(in_maps, trace=False)


# revision 16
# speedup vs baseline: 1.3847x; 1.3587x over previous
"""Trainium2 Bass kernel for nn_AttentionLayer (sparse euclidean attention).

Math (reference):
    a      = tanh(attended @ W_A_X) + b_A_X          [L, D]
    M[i,j] = sum_d W_A[d] * (a[j,d] - a[i,d])^2      (>=0, 0 on diagonal)
    energy = softmax(-M, axis=1)
    glimpsed = energy @ source
    out    = tanh(concat([glimpsed, source]) @ W_A_combine) + b_A_combine

Rewrite used here: with b = a * W_A and wsq[j] = a[j]·b[j],
    -M[i,j] = 2*a_i·b_j - wsq_i - wsq_j.
wsq_i is constant per row and cancels in the softmax, so per query q:
    E'[k,q]   = exp(2*(a_q·b_k) - wsq_k - CSHIFT)
    energy    = E' / sum_k E'
No row-max pass is needed: the true max logit sits at k=q (M=0), and with
CSHIFT=40 every quantity stays comfortably inside fp32 (wsq is ~[36, 61]
for the target distribution; safe for wsq up to ~125).

Distribution: queries (rows) sharded 8 ways; every core holds the full
key-side tensors (b^T, wsq, source) which it computes itself from the
replicated attended^T input. No collectives.

Layouts (per core, transposed so no on-chip transposes are ever needed):
    mm1:  S^T[k,q]  = bT[d,k].T @ aqT[d,q]     (PSUM [128k, 512q])
    exp:  E'^T[k,q] = Exp(2*S^T + bias[k])     (ACT, per-partition bias)
    mm2:  G^T[s,q] += srcN[k,s].T @ E'^T[k,q]  (accumulate over 64 k-tiles)
    den:  den_acc[128,q] += E'^T  on DVE; one ones-matmul at the end
          replicates the partition-sum, reciprocal_approx_fast inverts.
    comb: out^T[s',q] = tanh(Wc[c,s'].T @ [G^T/den ; srcT_q]) + b_c

v4 notes:
- Engine queues are FIFO in emission order, so the prologue and main
  loop are software-pipelined at EMISSION level: l-tiles 8-15 are
  interleaved with main-loop k-tiles 0-23 of q-tile 0 (3 per l-tile).
  All cross-phase tensors (attT, bT, srcN, wsqT, negrow) are chunked
  into separate tiles so dependencies resolve per-chunk.
- den removed from PE (was 64x 307ns [128,1,512] matmuls per q-tile)
  and accumulated on DVE (gpsimd measured 3x slower than DVE for
  [128,512] tensor ops - keep it to memsets/DMA triggers only).
- wsq row->bias transpose via one DRAM round trip (512B-contiguous
  descriptors) + PE transpose per half, not a 16K 4B-descriptor
  scatter.
- All DRAM inputs host-prepped partition-major; a few big DMAs split
  between the two HWDGE rings (sync + scalar) in priority order.
"""

import numpy as np

L = 8192
D = 256
S = 256
NCORES = 8
Q = L // NCORES          # 1024 queries per core
KT = 128                 # key tile (PSUM partition dim)
NK = L // KT             # 64 key tiles
LT = 512                 # prologue l-tile width
NL = L // LT             # 16 prologue tiles
QT = 512                 # query tile (PSUM free dim)
NQ = Q // QT             # 2 query tiles per core
CSHIFT = 40.0
DELAY = 5                # k-tiles mm1/exp run ahead of mm2
PRO_STEPS = 3            # main-loop k-tiles interleaved per late l-tile

_cache = {}


def _build():
    import concourse.bass as bass
    import concourse.tile as tile
    from concourse import bacc, mybir, masks

    F32 = mybir.dt.float32
    BF16 = mybir.dt.bfloat16
    AF = mybir.ActivationFunctionType
    ALU = mybir.AluOpType

    nc = bacc.Bacc("TRN2", target_bir_lowering=False, debug=False)

    # host-prepped, partition-major DRAM inputs
    attT_d = nc.dram_tensor("attT", [4, 128, 2, 2048], BF16, kind="ExternalInput")
    attTq_d = nc.dram_tensor("attTq", [128, 2, Q], BF16, kind="ExternalInput")
    srcN_d = nc.dram_tensor("srcN", [2, 128, 32, S], BF16, kind="ExternalInput")
    srcTq_d = nc.dram_tensor("srcTq", [128, 2, Q], BF16, kind="ExternalInput")
    waxT_d = nc.dram_tensor("waxT", [128, 2, 2, 128], BF16, kind="ExternalInput")
    wc_d = nc.dram_tensor("wc", [128, 4, 2, 128], BF16, kind="ExternalInput")
    smalls_d = nc.dram_tensor("smalls", [128, 8], F32, kind="ExternalInput")
    nwabf_d = nc.dram_tensor("nwabf", [128, 2], BF16, kind="ExternalInput")
    out_d = nc.dram_tensor("out", [2, 128, Q], F32, kind="ExternalOutput")

    with tile.TileContext(nc) as tc:
        with tc.tile_pool(name="persist", bufs=1) as persist:
            # chunked cross-phase tensors (separate tiles => per-chunk deps)
            bT = [persist.tile([128, 2, 2048], BF16, tag=f"bT{i}", name=f"bT{i}")
                  for i in range(4)]
            attT_sb = [persist.tile([128, 2, 2048], BF16, tag=f"attT{i}",
                                    name=f"attT{i}") for i in range(4)]
            srcN_sb = [persist.tile([128, 32, S], BF16, tag=f"srcN{i}",
                                    name=f"srcN{i}") for i in range(2)]
            negrow = [persist.tile([1, 4096], F32, tag=f"negrow{i}",
                                   name=f"negrow{i}") for i in range(2)]
            wsq64 = [persist.tile([32, 128], F32, tag=f"wsq64_{i}",
                                  name=f"wsq64_{i}") for i in range(2)]
            wsqT = [persist.tile([128, 32], F32, tag=f"wsqT{i}",
                                 name=f"wsqT{i}") for i in range(2)]
            aq = persist.tile([128, 2, Q], BF16, tag="aq")
            srcTq_sb = persist.tile([128, 2, Q], BF16, tag="srcTq")
            attTq_sb = persist.tile([128, 2, Q], BF16, tag="attTq")
            waxT_sb = persist.tile([128, 2, 2, 128], BF16, tag="waxT")
            wc_sb = persist.tile([128, 4, 2, 128], BF16, tag="wc")
            smalls_sb = persist.tile([128, 8], F32, tag="smalls")
            nwabf_sb = persist.tile([128, 2], BF16, tag="nwabf")
            ones_sb = persist.tile([128, 128], BF16, tag="ones")
            ident_sb = persist.tile([32, 32], F32, tag="ident")

            bax = [smalls_sb[:, c:c + 1] for c in range(2)]
            wa = [smalls_sb[:, 2 + c:3 + c] for c in range(2)]
            bac = [smalls_sb[:, 4 + c:5 + c] for c in range(2)]
            waxb = [smalls_sb[:, 6 + c:7 + c] for c in range(2)]  # wa*bax

            # --- input DMAs: sync ring gets the attended stream, scalar
            # ring the source stream; FIFO order per ring = priority ---
            nc.sync.dma_start(out=waxT_sb[:], in_=waxT_d[:])
            nc.sync.dma_start(out=attTq_sb[:], in_=attTq_d[:])
            nc.scalar.dma_start(out=smalls_sb[:], in_=smalls_d[:])
            nc.scalar.dma_start(out=nwabf_sb[:], in_=nwabf_d[:])
            for t in range(4):
                nc.sync.dma_start(out=attT_sb[t][:], in_=attT_d[t])
            for hh in range(2):
                nc.scalar.dma_start(out=srcN_sb[hh][:], in_=srcN_d[hh])
            nc.scalar.dma_start(out=srcTq_sb[:], in_=srcTq_d[:])
            nc.scalar.dma_start(out=wc_sb[:], in_=wc_d[:])

            nc.vector.memset(ones_sb[:], 1.0)
            masks.make_identity(nc, ident_sb[:])

            with tc.tile_pool(name="dr", bufs=1, space="DRAM") as dr, \
                 tc.tile_pool(name="atq", bufs=2) as atq_p, \
                 tc.tile_pool(name="at", bufs=3) as at_p, \
                 tc.tile_pool(name="sq", bufs=3) as sq_p, \
                 tc.tile_pool(name="eT", bufs=9) as eT_p, \
                 tc.tile_pool(name="gN", bufs=2) as gN_p, \
                 tc.tile_pool(name="ct", bufs=2) as ct_p, \
                 tc.tile_pool(name="rcp", bufs=2) as rcp_p, \
                 tc.tile_pool(name="dacc", bufs=2) as dacc_p, \
                 tc.tile_pool(name="psS", bufs=4, space="PSUM") as psS, \
                 tc.tile_pool(name="psW", bufs=1, space="PSUM") as psW, \
                 tc.tile_pool(name="psG", bufs=1, space="PSUM") as psG, \
                 tc.tile_pool(name="psD", bufs=1, space="PSUM") as psD:

                wsq_dram = dr.tile([L], F32, tag="wsq_dram")

                # ============ query transform: aq = a^T[:, own] ============
                for h in range(NQ):
                    for m in range(2):
                        ps = psS.tile([128, QT], F32, tag="s")
                        for c in range(2):
                            nc.tensor.matmul(
                                ps[:],
                                waxT_sb[:, c, m, :],
                                attTq_sb[:, c, h * QT:(h + 1) * QT],
                                start=(c == 0), stop=(c == 1),
                            )
                        at_q = atq_p.tile([128, QT], F32, tag="atq")
                        nc.scalar.activation(
                            out=at_q[:], in_=ps[:], func=AF.Tanh
                        )
                        nc.vector.tensor_scalar_add(
                            aq[:, m, h * QT:(h + 1) * QT], at_q[:],
                            bax[m],
                        )

                # ========== prologue l-tile bodies ==========
                # per l-tile: mma -> tanh(ACT) ->
                #   bT_c = tanh*wa + (wa*bax)   (DVE fused tensor_scalar)
                #   sq_0 = (tanh+bax)^2         (ACT Square with bias)
                #   sq_1 = (tanh+bax)^2         (DVE add, DVE mult)
                #   psW  = -sum_d W_A*sq        (PE rank-1, both chunks)
                mma_state = {}

                def emit_mma(t):
                    pss = []
                    for m in range(2):
                        ps = psS.tile([128, LT], F32, tag="s")
                        for c in range(2):
                            nc.tensor.matmul(
                                ps[:],
                                waxT_sb[:, c, m, :],
                                attT_sb[t // 4][:, c, (t % 4) * LT:(t % 4 + 1) * LT],
                                start=(c == 0), stop=(c == 1),
                            )
                        pss.append(ps)
                    return pss

                def emit_ltile(t):
                    ps_prev = mma_state.pop(t)
                    at_t = at_p.tile([128, 2, LT], F32, tag="at")
                    for m in range(2):
                        nc.scalar.activation(
                            out=at_t[:, m, :], in_=ps_prev[m][:], func=AF.Tanh,
                        )
                    sq_t = sq_p.tile([128, 2, LT], BF16, tag="sq")
                    cc = t // 4
                    ll = (t % 4) * LT
                    for c in range(2):
                        nc.vector.tensor_scalar(
                            bT[cc][:, c, ll:ll + LT], at_t[:, c, :],
                            wa[c], waxb[c],
                            op0=ALU.mult, op1=ALU.add,
                        )
                    nc.scalar.activation(
                        out=sq_t[:, 0, :], in_=at_t[:, 0, :],
                        func=AF.Square, bias=bax[0], scale=1.0,
                    )
                    at1 = at_p.tile([128, LT], F32, tag="at1")
                    nc.vector.tensor_scalar_add(at1[:], at_t[:, 1, :], bax[1])
                    nc.vector.tensor_tensor(
                        out=sq_t[:, 1, :], in0=at1[:], in1=at1[:], op=ALU.mult,
                    )
                    ps_w = psW.tile([1, LT], F32, tag="psW")
                    for c in range(2):
                        nc.tensor.matmul(
                            ps_w[:], nwabf_sb[:, c:c + 1], sq_t[:, c, :],
                            start=(c == 0), stop=(c == 1),
                        )
                    nc.vector.tensor_copy(
                        out=negrow[t // 8][0:1, (t % 8) * LT:(t % 8 + 1) * LT],
                        in_=ps_w[:],
                    )
                    if t + 1 < NL:
                        mma_state[t + 1] = emit_mma(t + 1)

                def finish_wsq_half(hh):
                    # negrow holds -wsq; round-trip through DRAM to get it
                    # partition-distributed, then PE-transpose [32,128] ->
                    # bias layout [128, 32].
                    nc.gpsimd.dma_start(
                        out=wsq_dram[hh * 4096:(hh + 1) * 4096],
                        in_=negrow[hh][0:1, :],
                    )
                    nc.gpsimd.dma_start(
                        out=wsq64[hh][:],
                        in_=bass.AP(
                            tensor=wsq_dram.tensor,
                            offset=wsq_dram.offset + hh * 4096,
                            ap=[[128, 32], [1, 128]],
                        ),
                    )
                    ps_t = psW.tile([128, 32], F32, tag="psW")
                    nc.tensor.transpose(ps_t[:], wsq64[hh][:], ident_sb[:])
                    nc.vector.tensor_scalar_add(wsqT[hh][:], ps_t[:], -CSHIFT)

                # ========== main-loop q-tile emission (step-driven) ==========
                def make_qtile(h):
                    aq0 = aq[:, 0, h * QT:(h + 1) * QT]
                    aq1 = aq[:, 1, h * QT:(h + 1) * QT]
                    ps_g = psG.tile([128, 2, QT], F32, tag="psG")
                    den = dacc_p.tile([128, QT], F32, tag="dacc")
                    st = {"ss": [], "es": [], "t": 0}

                    def emit_mm1(t):
                        ps_s = psS.tile([128, QT], F32, tag="s")
                        cc, kk = t // 16, (t % 16) * KT
                        nc.tensor.matmul(
                            ps_s[:], bT[cc][:, 0, kk:kk + KT], aq0,
                            start=True, stop=False,
                        )
                        nc.tensor.matmul(
                            ps_s[:], bT[cc][:, 1, kk:kk + KT], aq1,
                            start=False, stop=True,
                        )
                        return ps_s

                    def emit_exp(t):
                        e_t = eT_p.tile([128, QT], BF16, tag="eT")
                        nc.scalar.activation(
                            out=e_t[:], in_=st["ss"][t][:], func=AF.Exp,
                            bias=wsqT[t // 32][:, t % 32:t % 32 + 1],
                            scale=2.0,
                        )
                        return e_t

                    def prime():
                        st["ss"].append(emit_mm1(0))
                        for t in range(DELAY):
                            st["es"].append(emit_exp(t))
                            st["ss"].append(emit_mm1(t + 1))

                    def step():
                        t = st["t"]
                        st["t"] = t + 1
                        es = st["es"]
                        if t + DELAY < NK:
                            es.append(emit_exp(t + DELAY))
                            st["ss"].append(
                                emit_mm1(t + DELAY + 1) if t + DELAY + 1 < NK
                                else None
                            )
                        for m in range(2):
                            nc.tensor.matmul(
                                ps_g[:, m, :],
                                srcN_sb[t // 32][:, t % 32, m * 128:(m + 1) * 128],
                                es[t][:],
                                start=(t == 0), stop=(t == NK - 1),
                            )
                        if t == 1:
                            nc.vector.tensor_tensor(
                                out=den[:], in0=es[0][:], in1=es[1][:],
                                op=ALU.add,
                            )
                        elif t > 1:
                            nc.vector.tensor_tensor(
                                out=den[:], in0=den[:], in1=es[t][:],
                                op=ALU.add,
                            )

                    def tail():
                        den_bf = rcp_p.tile([128, QT], BF16, tag="denbf")
                        nc.vector.tensor_copy(out=den_bf[:], in_=den[:])
                        ps_d = psD.tile([128, QT], F32, tag="psD")
                        nc.tensor.matmul(
                            ps_d[:], ones_sb[:], den_bf[:],
                            start=True, stop=True,
                        )
                        rcp = rcp_p.tile([128, QT], F32, tag="rcp")
                        nc.vector.reciprocal_approx_fast(out=rcp[:], in_=ps_d[:])
                        g_n = gN_p.tile([128, 2, QT], BF16, tag="gN")
                        for m in range(2):
                            nc.vector.tensor_tensor(
                                out=g_n[:, m, :], in0=ps_g[:, m, :],
                                in1=rcp[:], op=ALU.mult,
                            )
                        for m in range(2):
                            ps_c = psS.tile([128, QT], F32, tag="s")
                            rhss = [
                                srcTq_sb[:, 0, h * QT:(h + 1) * QT],
                                srcTq_sb[:, 1, h * QT:(h + 1) * QT],
                                g_n[:, 0, :], g_n[:, 1, :],
                            ]
                            cidx = [2, 3, 0, 1]
                            for j in range(4):
                                nc.tensor.matmul(
                                    ps_c[:], wc_sb[:, cidx[j], m, :], rhss[j],
                                    start=(j == 0), stop=(j == 3),
                                )
                            c_t = ct_p.tile([128, QT], F32, tag="ct")
                            nc.scalar.activation(
                                out=c_t[:], in_=ps_c[:], func=AF.Tanh
                            )
                            nc.vector.tensor_scalar_add(c_t[:], c_t[:], bac[m])
                            nc.sync.dma_start(
                                out=out_d[m, :, h * QT:(h + 1) * QT],
                                in_=c_t[:],
                            )
                    return prime, step, tail

                # ========== emission schedule ==========
                # l-tiles 0-7, wsq half 0, then l-tiles 8-15 interleaved
                # with the first PRO_STEPS*8 k-tiles of q-tile 0.
                mma_state[0] = emit_mma(0)
                for t in range(NL // 2):
                    emit_ltile(t)
                finish_wsq_half(0)

                prime0, step0, tail0 = make_qtile(0)
                prime0()
                for t in range(NL // 2, NL):
                    emit_ltile(t)
                    for _ in range(PRO_STEPS):
                        step0()
                finish_wsq_half(1)
                for _ in range(NK - PRO_STEPS * (NL // 2)):
                    step0()

                # qtile1's mm1 pipeline primes before tail0 so the PE keeps
                # streaming; tail0 must precede qtile1's first mm2 (psG WAR).
                prime1, step1, tail1 = make_qtile(1)
                prime1()
                tail0()
                for _ in range(NK):
                    step1()
                tail1()

    nc.compile()
    return nc


def _get_nc():
    if "nc" not in _cache:
        _cache["nc"] = _build()
    return _cache["nc"]


def _bf16(x):
    import ml_dtypes

    return np.ascontiguousarray(x, dtype=ml_dtypes.bfloat16)


def _prep_inputs(attended, source, W_A_X, b_A_X, W_A, W_A_combine, b_A_combine):
    f = np.float32
    att = np.asarray(attended, dtype=f)
    src = np.asarray(source, dtype=f)
    attT = _bf16(att.T.reshape(2, 128, 4, 2048).transpose(2, 1, 0, 3))
    srcN = _bf16(src.reshape(2, 32, 128, S).transpose(0, 2, 1, 3))
    waxT = _bf16(
        np.asarray(W_A_X, dtype=f).reshape(2, 128, 2, 128).transpose(1, 0, 2, 3)
    )
    wc = _bf16(
        np.asarray(W_A_combine, dtype=f).reshape(4, 128, 2, 128).transpose(1, 0, 2, 3)
    )
    bax = np.asarray(b_A_X, dtype=f)
    wa = np.asarray(W_A, dtype=f)
    bac = np.asarray(b_A_combine, dtype=f)
    smalls = np.ascontiguousarray(
        np.stack(
            [bax[:128], bax[128:], wa[:128], wa[128:], bac[:128], bac[128:],
             wa[:128] * bax[:128], wa[128:] * bax[128:]],
            axis=1,
        )
    )
    nwabf = _bf16((-wa).reshape(2, 128).T)

    in_maps = []
    for i in range(NCORES):
        sl = slice(i * Q, (i + 1) * Q)
        attTq = _bf16(att[sl].T.reshape(2, 128, Q).transpose(1, 0, 2))
        srcTq = _bf16(src[sl].T.reshape(2, 128, Q).transpose(1, 0, 2))
        in_maps.append({
            "attT": attT, "attTq": attTq, "srcN": srcN, "srcTq": srcTq,
            "waxT": waxT, "wc": wc, "smalls": smalls, "nwabf": nwabf,
        })
    return in_maps


def _run(in_maps, trace=False):
    from concourse.bass_utils import run_bass_kernel_spmd

    nc = _get_nc()
    res = run_bass_kernel_spmd(nc, in_maps, list(range(NCORES)), trace=trace)
    _cache["last_result"] = res
    out = np.empty((L, S), dtype=np.float32)
    for i in range(NCORES):
        o = res.results[i]["out"]          # [2, 128, Q] = combined^T chunks
        out[i * Q:(i + 1) * Q, :] = np.asarray(o, dtype=np.float32).reshape(S, Q).T
    return out


def kernel(**inputs):
    in_maps = _prep_inputs(**inputs)
    return _run(in_maps, trace=False)


# revision 21
# speedup vs baseline: 1.3947x; 1.0072x over previous
"""Trainium2 Bass kernel for nn_AttentionLayer (sparse euclidean attention).

Math (reference):
    a      = tanh(attended @ W_A_X) + b_A_X          [L, D]
    M[i,j] = sum_d W_A[d] * (a[j,d] - a[i,d])^2      (>=0, 0 on diagonal)
    energy = softmax(-M, axis=1)
    glimpsed = energy @ source
    out    = tanh(concat([glimpsed, source]) @ W_A_combine) + b_A_combine

Rewrite used here: with b = a * W_A and wsq[j] = a[j]·b[j],
    -M[i,j] = 2*a_i·b_j - wsq_i - wsq_j.
wsq_i is constant per row and cancels in the softmax, so per query q:
    E'[k,q]   = exp(2*(a_q·b_k) - wsq_k - CSHIFT)
    energy    = E' / sum_k E'
No row-max pass is needed: the true max logit sits at k=q (M=0), and with
CSHIFT=40 every quantity stays comfortably inside fp32 (wsq is ~[36, 61]
for the target distribution; safe for wsq up to ~125).

Distribution: queries (rows) sharded 8 ways; every core holds the full
key-side tensors (b^T, wsq, source) which it computes itself from the
replicated attended^T input. No collectives.

Layouts (per core, transposed so no on-chip transposes are ever needed):
    mm1:  S^T[k,q]  = bT[d,k].T @ aqT[d,q]     (PSUM [128k, 512q])
    exp:  E'^T[k,q] = Exp(2*S^T + bias[k])     (ACT, per-partition bias)
    mm2:  G^T[s,q] += srcN[k,s].T @ E'^T[k,q]  (accumulate over 64 k-tiles)
    den:  den_acc[128,q] += E'^T  on DVE; one ones-matmul at the end
          replicates the partition-sum, reciprocal_approx_fast inverts.
    comb: out^T[s',q] = tanh(Wc[c,s'].T @ [G^T/den ; srcT_q]) + b_c

v4 notes:
- Engine queues are FIFO in emission order, so the prologue and main
  loop are software-pipelined at EMISSION level: l-tiles 8-15 are
  interleaved with main-loop k-tiles 0-23 of q-tile 0 (3 per l-tile).
  All cross-phase tensors (attT, bT, srcN, wsqT, negrow) are chunked
  into separate tiles so dependencies resolve per-chunk.
- den removed from PE (was 64x 307ns [128,1,512] matmuls per q-tile)
  and accumulated on DVE (gpsimd measured 3x slower than DVE for
  [128,512] tensor ops - keep it to memsets/DMA triggers only).
- wsq row->bias transpose via one DRAM round trip (512B-contiguous
  descriptors) + PE transpose per half, not a 16K 4B-descriptor
  scatter.
- All DRAM inputs host-prepped partition-major; a few big DMAs split
  between the two HWDGE rings (sync + scalar) in priority order.
"""

import numpy as np

L = 8192
D = 256
S = 256
NCORES = 8
Q = L // NCORES          # 1024 queries per core
KT = 128                 # key tile (PSUM partition dim)
NK = L // KT             # 64 key tiles
LT = 512                 # prologue l-tile width
NL = L // LT             # 16 prologue tiles
QT = 512                 # query tile (PSUM free dim)
NQ = Q // QT             # 2 query tiles per core
CSHIFT = 40.0
DELAY = 2                # k-tiles mm1/exp run ahead of mm2
PRO_STEPS = 4            # main-loop k-tiles interleaved per late l-tile

_cache = {}


def _build():
    import concourse.bass as bass
    import concourse.tile as tile
    from concourse import bacc, mybir, masks

    F32 = mybir.dt.float32
    BF16 = mybir.dt.bfloat16
    AF = mybir.ActivationFunctionType
    ALU = mybir.AluOpType

    nc = bacc.Bacc("TRN2", target_bir_lowering=False, debug=False)

    # host-prepped, partition-major DRAM inputs
    attT_d = nc.dram_tensor("attT", [4, 128, 2, 2048], BF16, kind="ExternalInput")
    attTq_d = nc.dram_tensor("attTq", [128, 2, Q], BF16, kind="ExternalInput")
    srcN_d = nc.dram_tensor("srcN", [2, 128, 32, S], BF16, kind="ExternalInput")
    srcTq_d = nc.dram_tensor("srcTq", [128, 2, Q], BF16, kind="ExternalInput")
    waxT_d = nc.dram_tensor("waxT", [128, 2, 2, 128], BF16, kind="ExternalInput")
    wc_d = nc.dram_tensor("wc", [128, 4, 2, 128], BF16, kind="ExternalInput")
    smalls_d = nc.dram_tensor("smalls", [128, 8], F32, kind="ExternalInput")
    nwabf_d = nc.dram_tensor("nwabf", [128, 2], BF16, kind="ExternalInput")
    out_d = nc.dram_tensor("out", [2, 128, Q], F32, kind="ExternalOutput")

    with tile.TileContext(nc) as tc:
        with tc.tile_pool(name="persist", bufs=1) as persist:
            # chunked cross-phase tensors (separate tiles => per-chunk deps)
            bT = [persist.tile([128, 2, 2048], BF16, tag=f"bT{i}", name=f"bT{i}")
                  for i in range(4)]
            attT_sb = [persist.tile([128, 2, 2048], BF16, tag=f"attT{i}",
                                    name=f"attT{i}") for i in range(4)]
            srcN_sb = [persist.tile([128, 32, S], BF16, tag=f"srcN{i}",
                                    name=f"srcN{i}") for i in range(2)]
            negrow = [persist.tile([1, 4096], F32, tag=f"negrow{i}",
                                   name=f"negrow{i}") for i in range(2)]
            wsq64 = [persist.tile([32, 128], F32, tag=f"wsq64_{i}",
                                  name=f"wsq64_{i}") for i in range(2)]
            wsqT = [persist.tile([128, 32], F32, tag=f"wsqT{i}",
                                 name=f"wsqT{i}") for i in range(2)]
            aq = persist.tile([128, 2, Q], BF16, tag="aq")
            srcTq_sb = persist.tile([128, 2, Q], BF16, tag="srcTq")
            attTq_sb = persist.tile([128, 2, Q], BF16, tag="attTq")
            waxT_sb = persist.tile([128, 2, 2, 128], BF16, tag="waxT")
            wc_sb = persist.tile([128, 4, 2, 128], BF16, tag="wc")
            smalls_sb = persist.tile([128, 8], F32, tag="smalls")
            nwabf_sb = persist.tile([128, 2], BF16, tag="nwabf")
            ones_sb = persist.tile([128, 128], BF16, tag="ones")
            ident_sb = persist.tile([32, 32], F32, tag="ident")

            bax = [smalls_sb[:, c:c + 1] for c in range(2)]
            wa = [smalls_sb[:, 2 + c:3 + c] for c in range(2)]
            bac = [smalls_sb[:, 4 + c:5 + c] for c in range(2)]
            waxb = [smalls_sb[:, 6 + c:7 + c] for c in range(2)]  # wa*bax

            # --- input DMAs: sync ring gets the attended stream, scalar
            # ring the source stream; FIFO order per ring = priority ---
            nc.sync.dma_start(out=attTq_sb[:], in_=attTq_d[:])
            nc.sync.dma_start(out=waxT_sb[:], in_=waxT_d[:])
            nc.scalar.dma_start(out=smalls_sb[:], in_=smalls_d[:])
            nc.scalar.dma_start(out=nwabf_sb[:], in_=nwabf_d[:])
            for t in range(4):
                nc.sync.dma_start(out=attT_sb[t][:], in_=attT_d[t])
            for hh in range(2):
                nc.scalar.dma_start(out=srcN_sb[hh][:], in_=srcN_d[hh])
            nc.scalar.dma_start(out=srcTq_sb[:], in_=srcTq_d[:])
            nc.scalar.dma_start(out=wc_sb[:], in_=wc_d[:])

            nc.vector.memset(ones_sb[:], 1.0)
            masks.make_identity(nc, ident_sb[:])

            with tc.tile_pool(name="dr", bufs=1, space="DRAM") as dr, \
                 tc.tile_pool(name="atq", bufs=2) as atq_p, \
                 tc.tile_pool(name="at", bufs=3) as at_p, \
                 tc.tile_pool(name="sq", bufs=3) as sq_p, \
                 tc.tile_pool(name="eT", bufs=9) as eT_p, \
                 tc.tile_pool(name="gN", bufs=2) as gN_p, \
                 tc.tile_pool(name="ct", bufs=2) as ct_p, \
                 tc.tile_pool(name="rcp", bufs=2) as rcp_p, \
                 tc.tile_pool(name="dacc", bufs=2) as dacc_p, \
                 tc.tile_pool(name="psS", bufs=4, space="PSUM") as psS, \
                 tc.tile_pool(name="psW", bufs=1, space="PSUM") as psW, \
                 tc.tile_pool(name="psG", bufs=1, space="PSUM") as psG, \
                 tc.tile_pool(name="psD", bufs=1, space="PSUM") as psD:

                wsq_dram = dr.tile([L], F32, tag="wsq_dram")

                # ============ query transform: aq = a^T[:, own] ============
                for h in range(NQ):
                    for m in range(2):
                        ps = psS.tile([128, QT], F32, tag="s")
                        for c in range(2):
                            nc.tensor.matmul(
                                ps[:],
                                waxT_sb[:, c, m, :],
                                attTq_sb[:, c, h * QT:(h + 1) * QT],
                                start=(c == 0), stop=(c == 1),
                            )
                        at_q = atq_p.tile([128, QT], F32, tag="atq")
                        nc.scalar.activation(
                            out=at_q[:], in_=ps[:], func=AF.Tanh
                        )
                        nc.vector.tensor_scalar_add(
                            aq[:, m, h * QT:(h + 1) * QT], at_q[:],
                            bax[m],
                        )

                # ========== prologue l-tile bodies ==========
                # per l-tile: mma -> tanh(ACT) ->
                #   bT_c = tanh*wa + (wa*bax)   (DVE fused tensor_scalar)
                #   sq_0 = (tanh+bax)^2         (ACT Square with bias)
                #   sq_1 = (tanh+bax)^2         (DVE add, DVE mult)
                #   psW  = -sum_d W_A*sq        (PE rank-1, both chunks)
                mma_state = {}

                def emit_mma(t):
                    pss = []
                    for m in range(2):
                        ps = psS.tile([128, LT], F32, tag="s")
                        for c in range(2):
                            nc.tensor.matmul(
                                ps[:],
                                waxT_sb[:, c, m, :],
                                attT_sb[t // 4][:, c, (t % 4) * LT:(t % 4 + 1) * LT],
                                start=(c == 0), stop=(c == 1),
                            )
                        pss.append(ps)
                    return pss

                def emit_ltile(t, sq1_act=False):
                    ps_prev = mma_state.pop(t)
                    at_t = at_p.tile([128, 2, LT], F32, tag="at")
                    for m in range(2):
                        nc.scalar.activation(
                            out=at_t[:, m, :], in_=ps_prev[m][:], func=AF.Tanh,
                        )
                    sq_t = sq_p.tile([128, 2, LT], BF16, tag="sq")
                    cc = t // 4
                    ll = (t % 4) * LT
                    for c in range(2):
                        nc.vector.tensor_scalar(
                            bT[cc][:, c, ll:ll + LT], at_t[:, c, :],
                            wa[c], waxb[c],
                            op0=ALU.mult, op1=ALU.add,
                        )
                    nc.scalar.activation(
                        out=sq_t[:, 0, :], in_=at_t[:, 0, :],
                        func=AF.Square, bias=bax[0], scale=1.0,
                    )
                    if sq1_act:
                        nc.scalar.activation(
                            out=sq_t[:, 1, :], in_=at_t[:, 1, :],
                            func=AF.Square, bias=bax[1], scale=1.0,
                        )
                    else:
                        at1 = at_p.tile([128, LT], F32, tag="at1")
                        nc.vector.tensor_scalar_add(
                            at1[:], at_t[:, 1, :], bax[1]
                        )
                        nc.vector.tensor_tensor(
                            out=sq_t[:, 1, :], in0=at1[:], in1=at1[:],
                            op=ALU.mult,
                        )
                    ps_w = psW.tile([1, LT], F32, tag="psW")
                    for c in range(2):
                        nc.tensor.matmul(
                            ps_w[:], nwabf_sb[:, c:c + 1], sq_t[:, c, :],
                            start=(c == 0), stop=(c == 1),
                        )
                    nc.vector.tensor_copy(
                        out=negrow[t // 8][0:1, (t % 8) * LT:(t % 8 + 1) * LT],
                        in_=ps_w[:],
                    )
                    if t + 1 < NL:
                        mma_state[t + 1] = emit_mma(t + 1)

                def wsq_dmas(hh, eng):
                    # negrow holds -wsq; round-trip through DRAM to get it
                    # partition-distributed (HWDGE ring, 512B-contiguous
                    # descriptors both ways)
                    eng.dma_start(
                        out=wsq_dram[hh * 4096:(hh + 1) * 4096],
                        in_=negrow[hh][0:1, :],
                    )
                    eng.dma_start(
                        out=wsq64[hh][:],
                        in_=bass.AP(
                            tensor=wsq_dram.tensor,
                            offset=wsq_dram.offset + hh * 4096,
                            ap=[[128, 32], [1, 128]],
                        ),
                    )

                def wsq_transpose(hh):
                    # PE-transpose [32,128] -> bias layout [128, 32]; emitted
                    # a couple of l-tiles after wsq_dmas so the PE never
                    # stalls on the DMA round trip.
                    ps_t = psW.tile([128, 32], F32, tag="psW")
                    nc.tensor.transpose(ps_t[:], wsq64[hh][:], ident_sb[:])
                    nc.vector.tensor_scalar_add(wsqT[hh][:], ps_t[:], -CSHIFT)

                # ========== main-loop q-tile emission (step-driven) ==========
                def make_qtile(h):
                    aq0 = aq[:, 0, h * QT:(h + 1) * QT]
                    aq1 = aq[:, 1, h * QT:(h + 1) * QT]
                    ps_g = psG.tile([128, 2, QT], F32, tag="psG")
                    den = dacc_p.tile([128, QT], F32, tag="dacc")
                    st = {"ss": [], "es": [], "t": 0}

                    def emit_mm1(t):
                        ps_s = psS.tile([128, QT], F32, tag="s")
                        cc, kk = t // 16, (t % 16) * KT
                        nc.tensor.matmul(
                            ps_s[:], bT[cc][:, 0, kk:kk + KT], aq0,
                            start=True, stop=False,
                        )
                        nc.tensor.matmul(
                            ps_s[:], bT[cc][:, 1, kk:kk + KT], aq1,
                            start=False, stop=True,
                        )
                        return ps_s

                    def emit_exp(t):
                        e_t = eT_p.tile([128, QT], BF16, tag="eT")
                        nc.scalar.activation(
                            out=e_t[:], in_=st["ss"][t][:], func=AF.Exp,
                            bias=wsqT[t // 32][:, t % 32:t % 32 + 1],
                            scale=2.0,
                        )
                        return e_t

                    def prime():
                        st["ss"].append(emit_mm1(0))
                        for t in range(DELAY):
                            st["es"].append(emit_exp(t))
                            st["ss"].append(emit_mm1(t + 1))

                    def step():
                        t = st["t"]
                        st["t"] = t + 1
                        es = st["es"]
                        if t + DELAY < NK:
                            es.append(emit_exp(t + DELAY))
                            st["ss"].append(
                                emit_mm1(t + DELAY + 1) if t + DELAY + 1 < NK
                                else None
                            )
                        for m in range(2):
                            nc.tensor.matmul(
                                ps_g[:, m, :],
                                srcN_sb[t // 32][:, t % 32, m * 128:(m + 1) * 128],
                                es[t][:],
                                start=(t == 0), stop=(t == NK - 1),
                            )
                        if t == 1:
                            nc.vector.tensor_tensor(
                                out=den[:], in0=es[0][:], in1=es[1][:],
                                op=ALU.add,
                            )
                        elif t > 1:
                            nc.vector.tensor_tensor(
                                out=den[:], in0=den[:], in1=es[t][:],
                                op=ALU.add,
                            )

                    def tail():
                        den_bf = rcp_p.tile([128, QT], BF16, tag="denbf")
                        nc.vector.tensor_copy(out=den_bf[:], in_=den[:])
                        ps_d = psD.tile([128, QT], F32, tag="psD")
                        nc.tensor.matmul(
                            ps_d[:], ones_sb[:], den_bf[:],
                            start=True, stop=True,
                        )
                        rcp = rcp_p.tile([128, QT], F32, tag="rcp")
                        nc.vector.reciprocal_approx_fast(out=rcp[:], in_=ps_d[:])
                        g_n = gN_p.tile([128, 2, QT], BF16, tag="gN")
                        for m in range(2):
                            nc.vector.tensor_tensor(
                                out=g_n[:, m, :], in0=ps_g[:, m, :],
                                in1=rcp[:], op=ALU.mult,
                            )
                        for m in range(2):
                            ps_c = psS.tile([128, QT], F32, tag="s")
                            rhss = [
                                srcTq_sb[:, 0, h * QT:(h + 1) * QT],
                                srcTq_sb[:, 1, h * QT:(h + 1) * QT],
                                g_n[:, 0, :], g_n[:, 1, :],
                            ]
                            cidx = [2, 3, 0, 1]
                            for j in range(4):
                                nc.tensor.matmul(
                                    ps_c[:], wc_sb[:, cidx[j], m, :], rhss[j],
                                    start=(j == 0), stop=(j == 3),
                                )
                            c_t = ct_p.tile([128, QT], F32, tag="ct")
                            nc.scalar.activation(
                                out=c_t[:], in_=ps_c[:], func=AF.Tanh
                            )
                            nc.vector.tensor_scalar_add(c_t[:], c_t[:], bac[m])
                            (nc.sync if m == 0 else nc.scalar).dma_start(
                                out=out_d[m, :, h * QT:(h + 1) * QT],
                                in_=c_t[:],
                            )
                    return prime, step, tail

                # ========== emission schedule ==========
                # phase 1: l-tiles 0-9 (sq1 alternates ACT/DVE); wsq half-0
                #   DMA round trip issued after l7, its PE transpose two
                #   l-tiles later so the PE never waits on it.
                # phase 2: l-tiles 10-15 interleaved with k-tiles 0-23 of
                #   q-tile 0 (4 per l-tile; exp lookahead DELAY=2 stays
                #   inside wsq half 0).
                # phase 3: wsq half-1 round trip covered by k-tiles 24-29,
                #   then the rest of q-tile 0 and all of q-tile 1.
                mma_state[0] = emit_mma(0)
                for t in range(8):
                    emit_ltile(t, sq1_act=(t % 2 == 0))
                wsq_dmas(0, nc.sync)
                emit_ltile(8, sq1_act=True)
                emit_ltile(9, sq1_act=False)
                wsq_transpose(0)

                prime0, step0, tail0 = make_qtile(0)
                prime0()
                for t in range(10, NL):
                    emit_ltile(t)
                    for _ in range(PRO_STEPS):
                        step0()
                wsq_dmas(1, nc.scalar)
                for _ in range(6):
                    step0()
                wsq_transpose(1)
                for _ in range(NK - PRO_STEPS * 6 - 6):
                    step0()

                # qtile1's mm1 pipeline primes before tail0 so the PE keeps
                # streaming; tail0 must precede qtile1's first mm2 (psG WAR).
                prime1, step1, tail1 = make_qtile(1)
                prime1()
                tail0()
                for _ in range(NK):
                    step1()
                tail1()

    nc.compile()
    return nc


def _get_nc():
    if "nc" not in _cache:
        _cache["nc"] = _build()
    return _cache["nc"]


def _bf16(x):
    import ml_dtypes

    return np.ascontiguousarray(x, dtype=ml_dtypes.bfloat16)


def _prep_inputs(attended, source, W_A_X, b_A_X, W_A, W_A_combine, b_A_combine):
    f = np.float32
    att = np.asarray(attended, dtype=f)
    src = np.asarray(source, dtype=f)
    attT = _bf16(att.T.reshape(2, 128, 4, 2048).transpose(2, 1, 0, 3))
    srcN = _bf16(src.reshape(2, 32, 128, S).transpose(0, 2, 1, 3))
    waxT = _bf16(
        np.asarray(W_A_X, dtype=f).reshape(2, 128, 2, 128).transpose(1, 0, 2, 3)
    )
    wc = _bf16(
        np.asarray(W_A_combine, dtype=f).reshape(4, 128, 2, 128).transpose(1, 0, 2, 3)
    )
    bax = np.asarray(b_A_X, dtype=f)
    wa = np.asarray(W_A, dtype=f)
    bac = np.asarray(b_A_combine, dtype=f)
    smalls = np.ascontiguousarray(
        np.stack(
            [bax[:128], bax[128:], wa[:128], wa[128:], bac[:128], bac[128:],
             wa[:128] * bax[:128], wa[128:] * bax[128:]],
            axis=1,
        )
    )
    nwabf = _bf16((-wa).reshape(2, 128).T)

    in_maps = []
    for i in range(NCORES):
        sl = slice(i * Q, (i + 1) * Q)
        attTq = _bf16(att[sl].T.reshape(2, 128, Q).transpose(1, 0, 2))
        srcTq = _bf16(src[sl].T.reshape(2, 128, Q).transpose(1, 0, 2))
        in_maps.append({
            "attT": attT, "attTq": attTq, "srcN": srcN, "srcTq": srcTq,
            "waxT": waxT, "wc": wc, "smalls": smalls, "nwabf": nwabf,
        })
    return in_maps


def _run(in_maps, trace=False):
    from concourse.bass_utils import run_bass_kernel_spmd

    nc = _get_nc()
    res = run_bass_kernel_spmd(nc, in_maps, list(range(NCORES)), trace=trace)
    _cache["last_result"] = res
    out = np.empty((L, S), dtype=np.float32)
    for i in range(NCORES):
        o = res.results[i]["out"]          # [2, 128, Q] = combined^T chunks
        out[i * Q:(i + 1) * Q, :] = np.asarray(o, dtype=np.float32).reshape(S, Q).T
    return out


def kernel(**inputs):
    in_maps = _prep_inputs(**inputs)
    return _run(in_maps, trace=False)


# revision 28
# speedup vs baseline: 1.4551x; 1.0433x over previous
"""Trainium2 Bass kernel for nn_AttentionLayer (sparse euclidean attention).

Math (reference):
    a      = tanh(attended @ W_A_X) + b_A_X          [L, D]
    M[i,j] = sum_d W_A[d] * (a[j,d] - a[i,d])^2      (>=0, 0 on diagonal)
    energy = softmax(-M, axis=1)
    glimpsed = energy @ source
    out    = tanh(concat([glimpsed, source]) @ W_A_combine) + b_A_combine

Rewrite used here: with b = a * W_A and wsq[j] = a[j]·b[j],
    -M[i,j] = 2*a_i·b_j - wsq_i - wsq_j.
wsq_i is constant per row and cancels in the softmax, so per query q:
    E'[k,q]   = exp(2*(a_q·b_k) - wsq_k - CSHIFT)
    energy    = E' / sum_k E'
No row-max pass is needed: the true max logit sits at k=q (M=0), and with
CSHIFT=40 every quantity stays comfortably inside fp32 (wsq is ~[36, 61]
for the target distribution; safe for wsq up to ~125).

Distribution: queries (rows) sharded 8 ways; every core holds the full
key-side tensors (b^T, wsq, source) which it computes itself from the
replicated attended^T input. No collectives.

Layouts (per core, transposed so no on-chip transposes are ever needed):
    mm1:  S^T[k,q]  = bT[d,k].T @ aqT[d,q]     (PSUM [128k, 512q])
    exp:  E'^T[k,q] = Exp(2*S^T + bias[k])     (ACT, per-partition bias)
    mm2:  G^T[s,q] += srcN[k,s].T @ E'^T[k,q]  (accumulate over 64 k-tiles)
    den:  den_acc[128,q] += E'^T  on DVE; one ones-matmul at the end
          replicates the partition-sum, reciprocal_approx_fast inverts.
    comb: out^T[s',q] = tanh(Wc[c,s'].T @ [G^T/den ; srcT_q]) + b_c

v4 notes:
- Engine queues are FIFO in emission order, so the prologue and main
  loop are software-pipelined at EMISSION level: l-tiles 8-15 are
  interleaved with main-loop k-tiles 0-23 of q-tile 0 (3 per l-tile).
  All cross-phase tensors (attT, bT, srcN, wsqT, negrow) are chunked
  into separate tiles so dependencies resolve per-chunk.
- den removed from PE (was 64x 307ns [128,1,512] matmuls per q-tile)
  and accumulated on DVE (gpsimd measured 3x slower than DVE for
  [128,512] tensor ops - keep it to memsets/DMA triggers only).
- wsq row->bias transpose via one DRAM round trip (512B-contiguous
  descriptors) + PE transpose per half, not a 16K 4B-descriptor
  scatter.
- All DRAM inputs host-prepped partition-major; a few big DMAs split
  between the two HWDGE rings (sync + scalar) in priority order.
"""

import numpy as np

L = 8192
D = 256
S = 256
NCORES = 8
Q = L // NCORES          # 1024 queries per core
KT = 128                 # key tile (PSUM partition dim)
NK = L // KT             # 64 key tiles
LT = 512                 # prologue l-tile width
NL = L // LT             # 16 prologue tiles
QT = 512                 # query tile (PSUM free dim)
NQ = Q // QT             # 2 query tiles per core
CSHIFT = 40.0
DELAY = 2                # k-tiles mm1/exp run ahead of mm2
PRO_STEPS = 2            # main-loop k-tiles interleaved per late l-tile

_cache = {}


def _build():
    import concourse.bass as bass
    import concourse.tile as tile
    from concourse import bacc, mybir, masks

    F32 = mybir.dt.float32
    BF16 = mybir.dt.bfloat16
    AF = mybir.ActivationFunctionType
    ALU = mybir.AluOpType

    nc = bacc.Bacc("TRN2", target_bir_lowering=False, debug=False)

    # host-prepped, partition-major DRAM inputs
    attT_d = nc.dram_tensor("attT", [4, 128, 2, 2048], BF16, kind="ExternalInput")
    attTq_d = nc.dram_tensor("attTq", [128, 2, Q], BF16, kind="ExternalInput")
    srcN_d = nc.dram_tensor("srcN", [2, 128, 32, S], BF16, kind="ExternalInput")
    srcTq_d = nc.dram_tensor("srcTq", [128, 2, Q], BF16, kind="ExternalInput")
    waxT_d = nc.dram_tensor("waxT", [128, 2, 2, 128], BF16, kind="ExternalInput")
    wc_d = nc.dram_tensor("wc", [128, 4, 2, 128], BF16, kind="ExternalInput")
    smalls_d = nc.dram_tensor("smalls", [128, 8], F32, kind="ExternalInput")
    out_d = nc.dram_tensor("out", [2, 128, Q], F32, kind="ExternalOutput")

    with tile.TileContext(nc) as tc:
        with tc.tile_pool(name="persist", bufs=1) as persist:
            # chunked cross-phase tensors (separate tiles => per-chunk deps)
            bT = [persist.tile([128, 2, 2048], BF16, tag=f"bT{i}", name=f"bT{i}")
                  for i in range(4)]
            attT_sb = [persist.tile([128, 2, 2048], BF16, tag=f"attT{i}",
                                    name=f"attT{i}") for i in range(4)]
            srcN_sb = [persist.tile([128, 32, S], BF16, tag=f"srcN{i}",
                                    name=f"srcN{i}") for i in range(2)]
            negrow = [persist.tile([1, 4096], F32, tag=f"negrow{i}",
                                   name=f"negrow{i}") for i in range(2)]
            wsq64 = [persist.tile([32, 128], F32, tag=f"wsq64_{i}",
                                  name=f"wsq64_{i}") for i in range(2)]
            wsqT = [persist.tile([128, 32], F32, tag=f"wsqT{i}",
                                 name=f"wsqT{i}") for i in range(2)]
            aq = persist.tile([128, 2, Q], BF16, tag="aq")
            srcTq_sb = persist.tile([128, 2, Q], BF16, tag="srcTq")
            attTq_sb = persist.tile([128, 2, Q], BF16, tag="attTq")
            waxT_sb = persist.tile([128, 2, 2, 128], BF16, tag="waxT")
            wc_sb = persist.tile([128, 4, 2, 128], BF16, tag="wc")
            smalls_sb = persist.tile([128, 8], F32, tag="smalls")
            negones_sb = persist.tile([128, 1], BF16, tag="negones")
            ones_sb = persist.tile([128, 128], BF16, tag="ones")
            ident_sb = persist.tile([32, 32], F32, tag="ident")

            bax = [smalls_sb[:, c:c + 1] for c in range(2)]
            swa = [smalls_sb[:, 2 + c:3 + c] for c in range(2)]   # sqrt(wa)
            bac = [smalls_sb[:, 4 + c:5 + c] for c in range(2)]
            swab = [smalls_sb[:, 6 + c:7 + c] for c in range(2)]  # sqrt(wa)*bax

            # --- input DMAs: sync ring gets the attended stream, scalar
            # ring the rest; srcN (not needed until the first mm2) goes
            # last so attT chunks get the HBM bandwidth first ---
            nc.sync.dma_start(out=attTq_sb[:], in_=attTq_d[:])
            nc.sync.dma_start(out=waxT_sb[:], in_=waxT_d[:])
            nc.scalar.dma_start(out=smalls_sb[:], in_=smalls_d[:])
            for t in range(4):
                nc.sync.dma_start(out=attT_sb[t][:], in_=attT_d[t])
            nc.scalar.dma_start(out=srcTq_sb[:], in_=srcTq_d[:])
            nc.scalar.dma_start(out=wc_sb[:], in_=wc_d[:])
            for hh in range(2):
                nc.scalar.dma_start(out=srcN_sb[hh][:], in_=srcN_d[hh])

            nc.vector.memset(ones_sb[:], 1.0)
            nc.vector.memset(negones_sb[:], -1.0)
            masks.make_identity(nc, ident_sb[:])

            with tc.tile_pool(name="dr", bufs=1, space="DRAM") as dr, \
                 tc.tile_pool(name="atq", bufs=2) as atq_p, \
                 tc.tile_pool(name="at", bufs=3) as at_p, \
                 tc.tile_pool(name="sq", bufs=3) as sq_p, \
                 tc.tile_pool(name="eT", bufs=9) as eT_p, \
                 tc.tile_pool(name="gN", bufs=2) as gN_p, \
                 tc.tile_pool(name="ct", bufs=2) as ct_p, \
                 tc.tile_pool(name="rcp", bufs=2) as rcp_p, \
                 tc.tile_pool(name="dacc", bufs=2) as dacc_p, \
                 tc.tile_pool(name="psS", bufs=4, space="PSUM") as psS, \
                 tc.tile_pool(name="psW", bufs=1, space="PSUM") as psW, \
                 tc.tile_pool(name="psG", bufs=1, space="PSUM") as psG, \
                 tc.tile_pool(name="psD", bufs=1, space="PSUM") as psD:

                wsq_dram = dr.tile([L], F32, tag="wsq_dram")

                # ============ query transform: aq = a^T[:, own] ============
                for h in range(NQ):
                    for m in range(2):
                        ps = psS.tile([128, QT], F32, tag="s")
                        for c in range(2):
                            nc.tensor.matmul(
                                ps[:],
                                waxT_sb[:, c, m, :],
                                attTq_sb[:, c, h * QT:(h + 1) * QT],
                                start=(c == 0), stop=(c == 1),
                            )
                        at_q = atq_p.tile([128, QT], F32, tag="atq")
                        nc.scalar.activation(
                            out=at_q[:], in_=ps[:], func=AF.Tanh
                        )
                        nc.vector.tensor_scalar(
                            aq[:, m, h * QT:(h + 1) * QT], at_q[:],
                            bax[m], swa[m],
                            op0=ALU.add, op1=ALU.mult,
                        )

                # ========== prologue l-tile bodies ==========
                # per l-tile: mma -> tanh(ACT) ->
                #   bT_c = tanh*wa + (wa*bax)   (DVE fused tensor_scalar)
                #   sq_0 = (tanh+bax)^2         (ACT Square with bias)
                #   sq_1 = (tanh+bax)^2         (DVE add, DVE mult)
                #   psW  = -sum_d W_A*sq        (PE rank-1, both chunks)
                mma_state = {}

                def emit_mma(t):
                    pss = []
                    for m in range(2):
                        ps = psS.tile([128, LT], F32, tag="s")
                        for c in range(2):
                            nc.tensor.matmul(
                                ps[:],
                                waxT_sb[:, c, m, :],
                                attT_sb[t // 4][:, c, (t % 4) * LT:(t % 4 + 1) * LT],
                                start=(c == 0), stop=(c == 1),
                            )
                        pss.append(ps)
                    return pss

                def emit_ltile(t):
                    # bT' = sqrt(wa)*(tanh+bax) on DVE (c0) / ACT-Identity
                    # (c1); sq = bT'*bT' in one DVE 16-bit op; the wsq
                    # reduction is then a plain -ones matmul.
                    ps_prev = mma_state.pop(t)
                    at_t = at_p.tile([128, 2, LT], F32, tag="at")
                    for m in range(2):
                        nc.scalar.activation(
                            out=at_t[:, m, :], in_=ps_prev[m][:], func=AF.Tanh,
                        )
                    sq_t = sq_p.tile([128, 2, LT], BF16, tag="sq")
                    cc = t // 4
                    ll = (t % 4) * LT
                    nc.vector.tensor_scalar(
                        bT[cc][:, 0, ll:ll + LT], at_t[:, 0, :],
                        bax[0], swa[0],
                        op0=ALU.add, op1=ALU.mult,
                    )
                    nc.scalar.activation(
                        out=bT[cc][:, 1, ll:ll + LT], in_=at_t[:, 1, :],
                        func=AF.Identity, bias=swab[1], scale=swa[1],
                    )
                    nc.vector.tensor_tensor(
                        out=sq_t[:], in0=bT[cc][:, :, ll:ll + LT],
                        in1=bT[cc][:, :, ll:ll + LT], op=ALU.mult,
                    )
                    ps_w = psW.tile([1, LT], F32, tag="psW")
                    for c in range(2):
                        nc.tensor.matmul(
                            ps_w[:], negones_sb[:], sq_t[:, c, :],
                            start=(c == 0), stop=(c == 1),
                        )
                    nc.vector.tensor_copy(
                        out=negrow[t // 8][0:1, (t % 8) * LT:(t % 8 + 1) * LT],
                        in_=ps_w[:],
                    )
                    if t + 1 < NL:
                        mma_state[t + 1] = emit_mma(t + 1)

                def wsq_dmas(hh, eng):
                    # negrow holds -wsq; round-trip through DRAM to get it
                    # partition-distributed (HWDGE ring, 512B-contiguous
                    # descriptors both ways)
                    eng.dma_start(
                        out=wsq_dram[hh * 4096:(hh + 1) * 4096],
                        in_=negrow[hh][0:1, :],
                    )
                    eng.dma_start(
                        out=wsq64[hh][:],
                        in_=bass.AP(
                            tensor=wsq_dram.tensor,
                            offset=wsq_dram.offset + hh * 4096,
                            ap=[[128, 32], [1, 128]],
                        ),
                    )

                def wsq_transpose(hh):
                    # PE-transpose [32,128] -> bias layout [128, 32]; emitted
                    # a couple of l-tiles after wsq_dmas so the PE never
                    # stalls on the DMA round trip.
                    ps_t = psW.tile([128, 32], F32, tag="psW")
                    nc.tensor.transpose(ps_t[:], wsq64[hh][:], ident_sb[:])
                    nc.vector.tensor_scalar_add(wsqT[hh][:], ps_t[:], -CSHIFT)

                # ========== main-loop q-tile emission (step-driven) ==========
                def make_qtile(h):
                    aq0 = aq[:, 0, h * QT:(h + 1) * QT]
                    aq1 = aq[:, 1, h * QT:(h + 1) * QT]
                    ps_g = psG.tile([128, 2, QT], F32, tag="psG")
                    den = dacc_p.tile([128, QT], F32, tag="dacc")
                    st = {"ss": [], "es": [], "t": 0}

                    def emit_mm1(t):
                        ps_s = psS.tile([128, QT], F32, tag="s")
                        cc, kk = t // 16, (t % 16) * KT
                        nc.tensor.matmul(
                            ps_s[:], bT[cc][:, 0, kk:kk + KT], aq0,
                            start=True, stop=False,
                        )
                        nc.tensor.matmul(
                            ps_s[:], bT[cc][:, 1, kk:kk + KT], aq1,
                            start=False, stop=True,
                        )
                        return ps_s

                    def emit_exp(t):
                        e_t = eT_p.tile([128, QT], BF16, tag="eT")
                        nc.scalar.activation(
                            out=e_t[:], in_=st["ss"][t][:], func=AF.Exp,
                            bias=wsqT[t // 32][:, t % 32:t % 32 + 1],
                            scale=2.0,
                        )
                        return e_t

                    def prime():
                        st["ss"].append(emit_mm1(0))
                        for t in range(DELAY):
                            st["es"].append(emit_exp(t))
                            st["ss"].append(emit_mm1(t + 1))

                    def step():
                        t = st["t"]
                        st["t"] = t + 1
                        es = st["es"]
                        if t + DELAY < NK:
                            es.append(emit_exp(t + DELAY))
                            st["ss"].append(
                                emit_mm1(t + DELAY + 1) if t + DELAY + 1 < NK
                                else None
                            )
                        for m in range(2):
                            nc.tensor.matmul(
                                ps_g[:, m, :],
                                srcN_sb[t // 32][:, t % 32, m * 128:(m + 1) * 128],
                                es[t][:],
                                start=(t == 0), stop=(t == NK - 1),
                            )
                        if t == 1:
                            nc.vector.tensor_tensor(
                                out=den[:], in0=es[0][:], in1=es[1][:],
                                op=ALU.add,
                            )
                        elif t > 1:
                            nc.vector.tensor_tensor(
                                out=den[:], in0=den[:], in1=es[t][:],
                                op=ALU.add,
                            )

                    def tail():
                        den_bf = rcp_p.tile([128, QT], BF16, tag="denbf")
                        nc.vector.tensor_copy(out=den_bf[:], in_=den[:])
                        ps_d = psD.tile([128, QT], F32, tag="psD")
                        nc.tensor.matmul(
                            ps_d[:], ones_sb[:], den_bf[:],
                            start=True, stop=True,
                        )
                        rcp = rcp_p.tile([128, QT], F32, tag="rcp")
                        nc.vector.reciprocal_approx_fast(out=rcp[:], in_=ps_d[:])
                        g_n = gN_p.tile([128, 2, QT], BF16, tag="gN")
                        for m in range(2):
                            nc.vector.tensor_tensor(
                                out=g_n[:, m, :], in0=ps_g[:, m, :],
                                in1=rcp[:], op=ALU.mult,
                            )
                        for m in range(2):
                            ps_c = psS.tile([128, QT], F32, tag="s")
                            rhss = [
                                srcTq_sb[:, 0, h * QT:(h + 1) * QT],
                                srcTq_sb[:, 1, h * QT:(h + 1) * QT],
                                g_n[:, 0, :], g_n[:, 1, :],
                            ]
                            cidx = [2, 3, 0, 1]
                            for j in range(4):
                                nc.tensor.matmul(
                                    ps_c[:], wc_sb[:, cidx[j], m, :], rhss[j],
                                    start=(j == 0), stop=(j == 3),
                                )
                            c_t = ct_p.tile([128, QT], F32, tag="ct")
                            nc.scalar.activation(
                                out=c_t[:], in_=ps_c[:], func=AF.Tanh
                            )
                            nc.vector.tensor_scalar_add(c_t[:], c_t[:], bac[m])
                            (nc.sync if m == 0 else nc.scalar).dma_start(
                                out=out_d[m, :, h * QT:(h + 1) * QT],
                                in_=c_t[:],
                            )
                    return prime, step, tail

                # ========== emission schedule ==========
                # phase 1: l-tiles 0-9 (sq1 alternates ACT/DVE); wsq half-0
                #   DMA round trip issued after l7, its PE transpose two
                #   l-tiles later so the PE never waits on it.
                # phase 2: l-tiles 10-15 interleaved with k-tiles 0-23 of
                #   q-tile 0 (4 per l-tile; exp lookahead DELAY=2 stays
                #   inside wsq half 0).
                # phase 3: wsq half-1 round trip covered by k-tiles 24-29,
                #   then the rest of q-tile 0 and all of q-tile 1.
                mma_state[0] = emit_mma(0)
                for t in range(8):
                    emit_ltile(t)
                wsq_dmas(0, nc.gpsimd)
                emit_ltile(8)
                emit_ltile(9)
                wsq_transpose(0)

                prime0, step0, tail0 = make_qtile(0)
                prime0()
                for t in range(10, NL):
                    emit_ltile(t)
                    for _ in range(PRO_STEPS):
                        step0()
                wsq_dmas(1, nc.scalar)
                for _ in range(6):
                    step0()
                wsq_transpose(1)
                for _ in range(NK - PRO_STEPS * 6 - 6):
                    step0()

                # qtile1's mm1 pipeline primes before tail0 so the PE keeps
                # streaming; tail0 must precede qtile1's first mm2 (psG WAR).
                prime1, step1, tail1 = make_qtile(1)
                prime1()
                tail0()
                for _ in range(NK):
                    step1()
                tail1()

    nc.compile()
    return nc


def _get_nc():
    if "nc" not in _cache:
        _cache["nc"] = _build()
    return _cache["nc"]


def _bf16(x):
    import ml_dtypes

    return np.ascontiguousarray(x, dtype=ml_dtypes.bfloat16)


def _prep_inputs(attended, source, W_A_X, b_A_X, W_A, W_A_combine, b_A_combine):
    f = np.float32
    att = np.asarray(attended, dtype=f)
    src = np.asarray(source, dtype=f)
    attT = _bf16(att.T.reshape(2, 128, 4, 2048).transpose(2, 1, 0, 3))
    srcN = _bf16(src.reshape(2, 32, 128, S).transpose(0, 2, 1, 3))
    waxT = _bf16(
        np.asarray(W_A_X, dtype=f).reshape(2, 128, 2, 128).transpose(1, 0, 2, 3)
    )
    wc = _bf16(
        np.asarray(W_A_combine, dtype=f).reshape(4, 128, 2, 128).transpose(1, 0, 2, 3)
    )
    bax = np.asarray(b_A_X, dtype=f)
    wa = np.asarray(W_A, dtype=f)
    swa = np.sqrt(wa)
    bac = np.asarray(b_A_combine, dtype=f)
    smalls = np.ascontiguousarray(
        np.stack(
            [bax[:128], bax[128:], swa[:128], swa[128:], bac[:128], bac[128:],
             swa[:128] * bax[:128], swa[128:] * bax[128:]],
            axis=1,
        )
    )

    in_maps = []
    for i in range(NCORES):
        sl = slice(i * Q, (i + 1) * Q)
        attTq = _bf16(att[sl].T.reshape(2, 128, Q).transpose(1, 0, 2))
        srcTq = _bf16(src[sl].T.reshape(2, 128, Q).transpose(1, 0, 2))
        in_maps.append({
            "attT": attT, "attTq": attTq, "srcN": srcN, "srcTq": srcTq,
            "waxT": waxT, "wc": wc, "smalls": smalls,
        })
    return in_maps


def _run(in_maps, trace=False):
    from concourse.bass_utils import run_bass_kernel_spmd

    nc = _get_nc()
    res = run_bass_kernel_spmd(nc, in_maps, list(range(NCORES)), trace=trace)
    _cache["last_result"] = res
    out = np.empty((L, S), dtype=np.float32)
    for i in range(NCORES):
        o = res.results[i]["out"]          # [2, 128, Q] = combined^T chunks
        out[i * Q:(i + 1) * Q, :] = np.asarray(o, dtype=np.float32).reshape(S, Q).T
    return out


def kernel(**inputs):
    in_maps = _prep_inputs(**inputs)
    return _run(in_maps, trace=False)


# revision 31
# speedup vs baseline: 1.4743x; 1.0132x over previous
"""Trainium2 Bass kernel for nn_AttentionLayer (sparse euclidean attention).

Math (reference):
    a      = tanh(attended @ W_A_X) + b_A_X          [L, D]
    M[i,j] = sum_d W_A[d] * (a[j,d] - a[i,d])^2      (>=0, 0 on diagonal)
    energy = softmax(-M, axis=1)
    glimpsed = energy @ source
    out    = tanh(concat([glimpsed, source]) @ W_A_combine) + b_A_combine

Rewrite used here: with b = a * W_A and wsq[j] = a[j]·b[j],
    -M[i,j] = 2*a_i·b_j - wsq_i - wsq_j.
wsq_i is constant per row and cancels in the softmax, so per query q:
    E'[k,q]   = exp(2*(a_q·b_k) - wsq_k - CSHIFT)
    energy    = E' / sum_k E'
No row-max pass is needed: the true max logit sits at k=q (M=0), and with
CSHIFT=40 every quantity stays comfortably inside fp32 (wsq is ~[36, 61]
for the target distribution; safe for wsq up to ~125).

Distribution: queries (rows) sharded 8 ways; every core holds the full
key-side tensors (b^T, wsq, source) which it computes itself from the
replicated attended^T input. No collectives.

Layouts (per core, transposed so no on-chip transposes are ever needed):
    mm1:  S^T[k,q]  = bT[d,k].T @ aqT[d,q]     (PSUM [128k, 512q])
    exp:  E'^T[k,q] = Exp(2*S^T + bias[k])     (ACT, per-partition bias)
    mm2:  G^T[s,q] += srcN[k,s].T @ E'^T[k,q]  (accumulate over 64 k-tiles)
    den:  den_acc[128,q] += E'^T  on DVE; one ones-matmul at the end
          replicates the partition-sum, reciprocal_approx_fast inverts.
    comb: out^T[s',q] = tanh(Wc[c,s'].T @ [G^T/den ; srcT_q]) + b_c

v4 notes:
- Engine queues are FIFO in emission order, so the prologue and main
  loop are software-pipelined at EMISSION level: l-tiles 8-15 are
  interleaved with main-loop k-tiles 0-23 of q-tile 0 (3 per l-tile).
  All cross-phase tensors (attT, bT, srcN, wsqT, negrow) are chunked
  into separate tiles so dependencies resolve per-chunk.
- den removed from PE (was 64x 307ns [128,1,512] matmuls per q-tile)
  and accumulated on DVE (gpsimd measured 3x slower than DVE for
  [128,512] tensor ops - keep it to memsets/DMA triggers only).
- wsq row->bias transpose via one DRAM round trip (512B-contiguous
  descriptors) + PE transpose per half, not a 16K 4B-descriptor
  scatter.
- All DRAM inputs host-prepped partition-major; a few big DMAs split
  between the two HWDGE rings (sync + scalar) in priority order.
"""

import numpy as np

L = 8192
D = 256
S = 256
NCORES = 8
Q = L // NCORES          # 1024 queries per core
KT = 128                 # key tile (PSUM partition dim)
NK = L // KT             # 64 key tiles
LT = 512                 # prologue l-tile width
NL = L // LT             # 16 prologue tiles
QT = 512                 # query tile (PSUM free dim)
NQ = Q // QT             # 2 query tiles per core
CSHIFT = 40.0
DELAY = 2                # k-tiles mm1/exp run ahead of mm2
PRO_STEPS = 2            # main-loop k-tiles interleaved per late l-tile

_cache = {}


def _build():
    import concourse.bass as bass
    import concourse.tile as tile
    from concourse import bacc, mybir, masks

    F32 = mybir.dt.float32
    BF16 = mybir.dt.bfloat16
    AF = mybir.ActivationFunctionType
    ALU = mybir.AluOpType

    nc = bacc.Bacc("TRN2", target_bir_lowering=False, debug=False)

    # host-prepped, partition-major DRAM inputs
    attT_d = nc.dram_tensor("attT", [4, 128, 2, 2048], BF16, kind="ExternalInput")
    attTq_d = nc.dram_tensor("attTq", [128, 2, Q], BF16, kind="ExternalInput")
    srcN_d = nc.dram_tensor("srcN", [2, 128, 32, S], BF16, kind="ExternalInput")
    srcTq_d = nc.dram_tensor("srcTq", [128, 2, Q], BF16, kind="ExternalInput")
    waxT_d = nc.dram_tensor("waxT", [128, 2, 2, 128], BF16, kind="ExternalInput")
    wc_d = nc.dram_tensor("wc", [128, 4, 2, 128], BF16, kind="ExternalInput")
    smalls_d = nc.dram_tensor("smalls", [128, 8], F32, kind="ExternalInput")
    out_d = nc.dram_tensor("out", [2, 128, Q], F32, kind="ExternalOutput")

    with tile.TileContext(nc) as tc:
        with tc.tile_pool(name="persist", bufs=1) as persist:
            # chunked cross-phase tensors (separate tiles => per-chunk deps)
            bT = [persist.tile([128, 2, 2048], BF16, tag=f"bT{i}", name=f"bT{i}")
                  for i in range(4)]
            attT_sb = [persist.tile([128, 2, 2048], BF16, tag=f"attT{i}",
                                    name=f"attT{i}") for i in range(4)]
            srcN_sb = [persist.tile([128, 32, S], BF16, tag=f"srcN{i}",
                                    name=f"srcN{i}") for i in range(2)]
            negrow = [persist.tile([1, 4096], F32, tag=f"negrow{i}",
                                   name=f"negrow{i}") for i in range(2)]
            wsq64 = [persist.tile([32, 128], F32, tag=f"wsq64_{i}",
                                  name=f"wsq64_{i}") for i in range(2)]
            wsqT = [persist.tile([128, 32], F32, tag=f"wsqT{i}",
                                 name=f"wsqT{i}") for i in range(2)]
            aq = persist.tile([128, 2, Q], BF16, tag="aq")
            srcTq_sb = persist.tile([128, 2, Q], BF16, tag="srcTq")
            attTq_sb = persist.tile([128, 2, Q], BF16, tag="attTq")
            waxT_sb = persist.tile([128, 2, 2, 128], BF16, tag="waxT")
            wc_sb = persist.tile([128, 4, 2, 128], BF16, tag="wc")
            smalls_sb = persist.tile([128, 8], F32, tag="smalls")
            negones_sb = persist.tile([128, 1], BF16, tag="negones")
            ones_sb = persist.tile([128, 128], BF16, tag="ones")
            ident_sb = persist.tile([32, 32], F32, tag="ident")

            bax = [smalls_sb[:, c:c + 1] for c in range(2)]
            swa = [smalls_sb[:, 2 + c:3 + c] for c in range(2)]   # sqrt(wa)
            bac = [smalls_sb[:, 4 + c:5 + c] for c in range(2)]
            swab = [smalls_sb[:, 6 + c:7 + c] for c in range(2)]  # sqrt(wa)*bax

            # --- input DMAs: ALL bulk on the sync ring in consumption
            # order (srcN after attT so attT chunks never wait behind it);
            # scalar ring takes the small tensors + later the wsq round
            # trips ---
            nc.sync.dma_start(out=attTq_sb[:], in_=attTq_d[:])
            nc.sync.dma_start(out=waxT_sb[:], in_=waxT_d[:])
            nc.scalar.dma_start(out=smalls_sb[:], in_=smalls_d[:])
            for t in range(4):
                nc.sync.dma_start(out=attT_sb[t][:], in_=attT_d[t])
            nc.scalar.dma_start(out=srcTq_sb[:], in_=srcTq_d[:])
            nc.scalar.dma_start(out=wc_sb[:], in_=wc_d[:])
            for hh in range(2):
                nc.sync.dma_start(out=srcN_sb[hh][:], in_=srcN_d[hh])

            nc.vector.memset(ones_sb[:], 1.0)
            nc.vector.memset(negones_sb[:], -1.0)
            masks.make_identity(nc, ident_sb[:])

            with tc.tile_pool(name="dr", bufs=1, space="DRAM") as dr, \
                 tc.tile_pool(name="atq", bufs=2) as atq_p, \
                 tc.tile_pool(name="at", bufs=3) as at_p, \
                 tc.tile_pool(name="sq", bufs=3) as sq_p, \
                 tc.tile_pool(name="eT", bufs=9) as eT_p, \
                 tc.tile_pool(name="gN", bufs=2) as gN_p, \
                 tc.tile_pool(name="ct", bufs=2) as ct_p, \
                 tc.tile_pool(name="rcp", bufs=2) as rcp_p, \
                 tc.tile_pool(name="dacc", bufs=2) as dacc_p, \
                 tc.tile_pool(name="psS", bufs=4, space="PSUM") as psS, \
                 tc.tile_pool(name="psW", bufs=1, space="PSUM") as psW, \
                 tc.tile_pool(name="psG", bufs=1, space="PSUM") as psG, \
                 tc.tile_pool(name="psD", bufs=1, space="PSUM") as psD:

                wsq_dram = dr.tile([L], F32, tag="wsq_dram")

                # ============ query transform: aq = a^T[:, own] ============
                for h in range(NQ):
                    for m in range(2):
                        ps = psS.tile([128, QT], F32, tag="s")
                        for c in range(2):
                            nc.tensor.matmul(
                                ps[:],
                                waxT_sb[:, c, m, :],
                                attTq_sb[:, c, h * QT:(h + 1) * QT],
                                start=(c == 0), stop=(c == 1),
                            )
                        at_q = atq_p.tile([128, QT], F32, tag="atq")
                        nc.scalar.activation(
                            out=at_q[:], in_=ps[:], func=AF.Tanh
                        )
                        nc.vector.tensor_scalar(
                            aq[:, m, h * QT:(h + 1) * QT], at_q[:],
                            bax[m], swa[m],
                            op0=ALU.add, op1=ALU.mult,
                        )

                # ========== prologue l-tile bodies ==========
                # per l-tile: mma -> tanh(ACT) ->
                #   bT_c = tanh*wa + (wa*bax)   (DVE fused tensor_scalar)
                #   sq_0 = (tanh+bax)^2         (ACT Square with bias)
                #   sq_1 = (tanh+bax)^2         (DVE add, DVE mult)
                #   psW  = -sum_d W_A*sq        (PE rank-1, both chunks)
                mma_state = {}

                def emit_mma(t):
                    pss = []
                    for m in range(2):
                        ps = psS.tile([128, LT], F32, tag="s")
                        for c in range(2):
                            nc.tensor.matmul(
                                ps[:],
                                waxT_sb[:, c, m, :],
                                attT_sb[t // 4][:, c, (t % 4) * LT:(t % 4 + 1) * LT],
                                start=(c == 0), stop=(c == 1),
                            )
                        pss.append(ps)
                    return pss

                def emit_ltile(t):
                    # bT' = sqrt(wa)*(tanh+bax) on DVE (c0) / ACT-Identity
                    # (c1); sq = bT'*bT' in one DVE 16-bit op; the wsq
                    # reduction is then a plain -ones matmul.
                    ps_prev = mma_state.pop(t)
                    at_t = at_p.tile([128, 2, LT], F32, tag="at")
                    for m in range(2):
                        nc.scalar.activation(
                            out=at_t[:, m, :], in_=ps_prev[m][:], func=AF.Tanh,
                        )
                    sq_t = sq_p.tile([128, 2, LT], BF16, tag="sq")
                    cc = t // 4
                    ll = (t % 4) * LT
                    nc.vector.tensor_scalar(
                        bT[cc][:, 0, ll:ll + LT], at_t[:, 0, :],
                        bax[0], swa[0],
                        op0=ALU.add, op1=ALU.mult,
                    )
                    nc.gpsimd.tensor_scalar(
                        bT[cc][:, 1, ll:ll + LT], at_t[:, 1, :],
                        bax[1], swa[1],
                        op0=ALU.add, op1=ALU.mult,
                    )
                    nc.vector.tensor_tensor(
                        out=sq_t[:], in0=bT[cc][:, :, ll:ll + LT],
                        in1=bT[cc][:, :, ll:ll + LT], op=ALU.mult,
                    )
                    ps_w = psW.tile([1, LT], F32, tag="psW")
                    for c in range(2):
                        nc.tensor.matmul(
                            ps_w[:], negones_sb[:], sq_t[:, c, :],
                            start=(c == 0), stop=(c == 1),
                        )
                    nc.vector.tensor_copy(
                        out=negrow[t // 8][0:1, (t % 8) * LT:(t % 8 + 1) * LT],
                        in_=ps_w[:],
                    )
                    if t + 1 < NL:
                        mma_state[t + 1] = emit_mma(t + 1)

                def wsq_dmas(hh, eng):
                    # negrow holds -wsq; round-trip through DRAM to get it
                    # partition-distributed (HWDGE ring, 512B-contiguous
                    # descriptors both ways)
                    eng.dma_start(
                        out=wsq_dram[hh * 4096:(hh + 1) * 4096],
                        in_=negrow[hh][0:1, :],
                    )
                    eng.dma_start(
                        out=wsq64[hh][:],
                        in_=bass.AP(
                            tensor=wsq_dram.tensor,
                            offset=wsq_dram.offset + hh * 4096,
                            ap=[[128, 32], [1, 128]],
                        ),
                    )

                def wsq_transpose(hh):
                    # PE-transpose [32,128] -> bias layout [128, 32]; emitted
                    # a couple of l-tiles after wsq_dmas so the PE never
                    # stalls on the DMA round trip.
                    ps_t = psW.tile([128, 32], F32, tag="psW")
                    nc.tensor.transpose(ps_t[:], wsq64[hh][:], ident_sb[:])
                    nc.vector.tensor_scalar_add(wsqT[hh][:], ps_t[:], -CSHIFT)

                # ========== main-loop q-tile emission (step-driven) ==========
                def make_qtile(h):
                    aq0 = aq[:, 0, h * QT:(h + 1) * QT]
                    aq1 = aq[:, 1, h * QT:(h + 1) * QT]
                    ps_g = psG.tile([128, 2, QT], F32, tag="psG")
                    den = dacc_p.tile([128, QT], F32, tag="dacc")
                    st = {"ss": [], "es": [], "t": 0}

                    def emit_mm1(t):
                        ps_s = psS.tile([128, QT], F32, tag="s")
                        cc, kk = t // 16, (t % 16) * KT
                        nc.tensor.matmul(
                            ps_s[:], bT[cc][:, 0, kk:kk + KT], aq0,
                            start=True, stop=False,
                        )
                        nc.tensor.matmul(
                            ps_s[:], bT[cc][:, 1, kk:kk + KT], aq1,
                            start=False, stop=True,
                        )
                        return ps_s

                    def emit_exp(t):
                        e_t = eT_p.tile([128, QT], BF16, tag="eT")
                        nc.scalar.activation(
                            out=e_t[:], in_=st["ss"][t][:], func=AF.Exp,
                            bias=wsqT[t // 32][:, t % 32:t % 32 + 1],
                            scale=2.0,
                        )
                        return e_t

                    def prime():
                        st["ss"].append(emit_mm1(0))
                        for t in range(DELAY):
                            st["es"].append(emit_exp(t))
                            st["ss"].append(emit_mm1(t + 1))

                    def step():
                        t = st["t"]
                        st["t"] = t + 1
                        es = st["es"]
                        if t + DELAY < NK:
                            es.append(emit_exp(t + DELAY))
                            st["ss"].append(
                                emit_mm1(t + DELAY + 1) if t + DELAY + 1 < NK
                                else None
                            )
                        for m in range(2):
                            nc.tensor.matmul(
                                ps_g[:, m, :],
                                srcN_sb[t // 32][:, t % 32, m * 128:(m + 1) * 128],
                                es[t][:],
                                start=(t == 0), stop=(t == NK - 1),
                            )
                        if t == 1:
                            nc.vector.tensor_tensor(
                                out=den[:], in0=es[0][:], in1=es[1][:],
                                op=ALU.add,
                            )
                        elif t > 1:
                            nc.vector.tensor_tensor(
                                out=den[:], in0=den[:], in1=es[t][:],
                                op=ALU.add,
                            )

                    def tail():
                        den_bf = rcp_p.tile([128, QT], BF16, tag="denbf")
                        nc.vector.tensor_copy(out=den_bf[:], in_=den[:])
                        ps_d = psD.tile([128, QT], F32, tag="psD")
                        nc.tensor.matmul(
                            ps_d[:], ones_sb[:], den_bf[:],
                            start=True, stop=True,
                        )
                        rcp = rcp_p.tile([128, QT], F32, tag="rcp")
                        nc.vector.reciprocal_approx_fast(out=rcp[:], in_=ps_d[:])
                        g_n = gN_p.tile([128, 2, QT], BF16, tag="gN")
                        for m in range(2):
                            nc.vector.tensor_tensor(
                                out=g_n[:, m, :], in0=ps_g[:, m, :],
                                in1=rcp[:], op=ALU.mult,
                            )
                        for m in range(2):
                            ps_c = psS.tile([128, QT], F32, tag="s")
                            rhss = [
                                srcTq_sb[:, 0, h * QT:(h + 1) * QT],
                                srcTq_sb[:, 1, h * QT:(h + 1) * QT],
                                g_n[:, 0, :], g_n[:, 1, :],
                            ]
                            cidx = [2, 3, 0, 1]
                            for j in range(4):
                                nc.tensor.matmul(
                                    ps_c[:], wc_sb[:, cidx[j], m, :], rhss[j],
                                    start=(j == 0), stop=(j == 3),
                                )
                            c_t = ct_p.tile([128, QT], F32, tag="ct")
                            nc.scalar.activation(
                                out=c_t[:], in_=ps_c[:], func=AF.Tanh
                            )
                            nc.vector.tensor_scalar_add(c_t[:], c_t[:], bac[m])
                            (nc.sync if m == 0 else nc.scalar).dma_start(
                                out=out_d[m, :, h * QT:(h + 1) * QT],
                                in_=c_t[:],
                            )
                    return prime, step, tail

                # ========== emission schedule ==========
                # phase 1: l-tiles 0-9 (sq1 alternates ACT/DVE); wsq half-0
                #   DMA round trip issued after l7, its PE transpose two
                #   l-tiles later so the PE never waits on it.
                # phase 2: l-tiles 10-15 interleaved with k-tiles 0-23 of
                #   q-tile 0 (4 per l-tile; exp lookahead DELAY=2 stays
                #   inside wsq half 0).
                # phase 3: wsq half-1 round trip covered by k-tiles 24-29,
                #   then the rest of q-tile 0 and all of q-tile 1.
                mma_state[0] = emit_mma(0)
                for t in range(8):
                    emit_ltile(t)
                wsq_dmas(0, nc.scalar)
                emit_ltile(8)
                emit_ltile(9)
                wsq_transpose(0)

                prime0, step0, tail0 = make_qtile(0)
                prime0()
                for t in range(10, NL):
                    emit_ltile(t)
                    for _ in range(PRO_STEPS):
                        step0()
                wsq_dmas(1, nc.scalar)
                for _ in range(6):
                    step0()
                wsq_transpose(1)
                for _ in range(NK - PRO_STEPS * 6 - 6):
                    step0()

                # qtile1's mm1 pipeline primes before tail0 so the PE keeps
                # streaming; tail0 must precede qtile1's first mm2 (psG WAR).
                prime1, step1, tail1 = make_qtile(1)
                prime1()
                tail0()
                for _ in range(NK):
                    step1()
                tail1()

    nc.compile()
    return nc


def _get_nc():
    if "nc" not in _cache:
        _cache["nc"] = _build()
    return _cache["nc"]


def _bf16(x):
    import ml_dtypes

    return np.ascontiguousarray(x, dtype=ml_dtypes.bfloat16)


def _prep_inputs(attended, source, W_A_X, b_A_X, W_A, W_A_combine, b_A_combine):
    f = np.float32
    att = np.asarray(attended, dtype=f)
    src = np.asarray(source, dtype=f)
    attT = _bf16(att.T.reshape(2, 128, 4, 2048).transpose(2, 1, 0, 3))
    srcN = _bf16(src.reshape(2, 32, 128, S).transpose(0, 2, 1, 3))
    waxT = _bf16(
        np.asarray(W_A_X, dtype=f).reshape(2, 128, 2, 128).transpose(1, 0, 2, 3)
    )
    wc = _bf16(
        np.asarray(W_A_combine, dtype=f).reshape(4, 128, 2, 128).transpose(1, 0, 2, 3)
    )
    bax = np.asarray(b_A_X, dtype=f)
    wa = np.asarray(W_A, dtype=f)
    swa = np.sqrt(wa)
    bac = np.asarray(b_A_combine, dtype=f)
    smalls = np.ascontiguousarray(
        np.stack(
            [bax[:128], bax[128:], swa[:128], swa[128:], bac[:128], bac[128:],
             swa[:128] * bax[:128], swa[128:] * bax[128:]],
            axis=1,
        )
    )

    in_maps = []
    for i in range(NCORES):
        sl = slice(i * Q, (i + 1) * Q)
        attTq = _bf16(att[sl].T.reshape(2, 128, Q).transpose(1, 0, 2))
        srcTq = _bf16(src[sl].T.reshape(2, 128, Q).transpose(1, 0, 2))
        in_maps.append({
            "attT": attT, "attTq": attTq, "srcN": srcN, "srcTq": srcTq,
            "waxT": waxT, "wc": wc, "smalls": smalls,
        })
    return in_maps


def _run(in_maps, trace=False):
    from concourse.bass_utils import run_bass_kernel_spmd

    nc = _get_nc()
    res = run_bass_kernel_spmd(nc, in_maps, list(range(NCORES)), trace=trace)
    _cache["last_result"] = res
    out = np.empty((L, S), dtype=np.float32)
    for i in range(NCORES):
        o = res.results[i]["out"]          # [2, 128, Q] = combined^T chunks
        out[i * Q:(i + 1) * Q, :] = np.asarray(o, dtype=np.float32).reshape(S, Q).T
    return out


def kernel(**inputs):
    in_maps = _prep_inputs(**inputs)
    return _run(in_maps, trace=False)


# revision 38
# speedup vs baseline: 1.4874x; 1.0089x over previous
"""Trainium2 Bass kernel for nn_AttentionLayer (sparse euclidean attention).

Math (reference):
    a      = tanh(attended @ W_A_X) + b_A_X          [L, D]
    M[i,j] = sum_d W_A[d] * (a[j,d] - a[i,d])^2      (>=0, 0 on diagonal)
    energy = softmax(-M, axis=1)
    glimpsed = energy @ source
    out    = tanh(concat([glimpsed, source]) @ W_A_combine) + b_A_combine

Rewrite used here: with b = a * W_A and wsq[j] = a[j]·b[j],
    -M[i,j] = 2*a_i·b_j - wsq_i - wsq_j.
wsq_i is constant per row and cancels in the softmax, so per query q:
    E'[k,q]   = exp(2*(a_q·b_k) - wsq_k - CSHIFT)
    energy    = E' / sum_k E'
No row-max pass is needed: the true max logit sits at k=q (M=0), and with
CSHIFT=40 every quantity stays comfortably inside fp32 (wsq is ~[36, 61]
for the target distribution; safe for wsq up to ~125).

Distribution: queries (rows) sharded 8 ways; every core holds the full
key-side tensors (b^T, wsq, source) which it computes itself from the
replicated attended^T input. No collectives.

Layouts (per core, transposed so no on-chip transposes are ever needed):
    mm1:  S^T[k,q]  = bT[d,k].T @ aqT[d,q]     (PSUM [128k, 512q])
    exp:  E'^T[k,q] = Exp(2*S^T + bias[k])     (ACT, per-partition bias)
    mm2:  G^T[s,q] += srcN[k,s].T @ E'^T[k,q]  (accumulate over 64 k-tiles)
    den:  den_acc[128,q] += E'^T  on DVE; one ones-matmul at the end
          replicates the partition-sum, reciprocal_approx_fast inverts.
    comb: out^T[s',q] = tanh(Wc[c,s'].T @ [G^T/den ; srcT_q]) + b_c

v4 notes:
- Engine queues are FIFO in emission order, so the prologue and main
  loop are software-pipelined at EMISSION level: l-tiles 8-15 are
  interleaved with main-loop k-tiles 0-23 of q-tile 0 (3 per l-tile).
  All cross-phase tensors (attT, bT, srcN, wsqT, negrow) are chunked
  into separate tiles so dependencies resolve per-chunk.
- den removed from PE (was 64x 307ns [128,1,512] matmuls per q-tile)
  and accumulated on DVE (gpsimd measured 3x slower than DVE for
  [128,512] tensor ops - keep it to memsets/DMA triggers only).
- wsq row->bias transpose via one DRAM round trip (512B-contiguous
  descriptors) + PE transpose per half, not a 16K 4B-descriptor
  scatter.
- All DRAM inputs host-prepped partition-major; a few big DMAs split
  between the two HWDGE rings (sync + scalar) in priority order.
"""

import numpy as np

L = 8192
D = 256
S = 256
NCORES = 8
Q = L // NCORES          # 1024 queries per core
KT = 128                 # key tile (PSUM partition dim)
NK = L // KT             # 64 key tiles
LT = 512                 # prologue l-tile width
NL = L // LT             # 16 prologue tiles
QT = 512                 # query tile (PSUM free dim)
NQ = Q // QT             # 2 query tiles per core
CSHIFT = 40.0
DELAY = 2                # k-tiles mm1/exp run ahead of mm2
PRO_STEPS = 2            # main-loop k-tiles interleaved per late l-tile

_cache = {}


def _build():
    import concourse.bass as bass
    import concourse.tile as tile
    from concourse import bacc, mybir, masks

    F32 = mybir.dt.float32
    BF16 = mybir.dt.bfloat16
    AF = mybir.ActivationFunctionType
    ALU = mybir.AluOpType

    nc = bacc.Bacc("TRN2", target_bir_lowering=False, debug=False)

    # host-prepped, partition-major DRAM inputs
    attT_d = nc.dram_tensor("attT", [4, 128, 2, 2048], BF16, kind="ExternalInput")
    attTq_d = nc.dram_tensor("attTq", [128, 2, Q], BF16, kind="ExternalInput")
    srcN_d = nc.dram_tensor("srcN", [2, 128, 32, S], BF16, kind="ExternalInput")
    srcTq_d = nc.dram_tensor("srcTq", [128, 2, Q], BF16, kind="ExternalInput")
    waxT_d = nc.dram_tensor("waxT", [128, 2, 2, 128], BF16, kind="ExternalInput")
    wc_d = nc.dram_tensor("wc", [128, 4, 2, 128], BF16, kind="ExternalInput")
    smalls_d = nc.dram_tensor("smalls", [128, 8], F32, kind="ExternalInput")
    out_d = nc.dram_tensor("out", [2, 128, Q], F32, kind="ExternalOutput")

    with tile.TileContext(nc) as tc:
        with tc.tile_pool(name="persist", bufs=1) as persist:
            # chunked cross-phase tensors (separate tiles => per-chunk deps)
            bT = [persist.tile([128, 2, 2048], BF16, tag=f"bT{i}", name=f"bT{i}")
                  for i in range(4)]
            attT_sb = [persist.tile([128, 2, 2048], BF16, tag=f"attT{i}",
                                    name=f"attT{i}") for i in range(4)]
            srcN_sb = [persist.tile([128, 32, S], BF16, tag=f"srcN{i}",
                                    name=f"srcN{i}") for i in range(2)]
            negrow = [persist.tile([1, 4096], F32, tag=f"negrow{i}",
                                   name=f"negrow{i}") for i in range(2)]
            wsq64 = [persist.tile([32, 128], F32, tag=f"wsq64_{i}",
                                  name=f"wsq64_{i}") for i in range(2)]
            wsqT = [persist.tile([128, 32], F32, tag=f"wsqT{i}",
                                 name=f"wsqT{i}") for i in range(2)]
            aq = persist.tile([128, 2, Q], BF16, tag="aq")
            srcTq_sb = persist.tile([128, 2, Q], BF16, tag="srcTq")
            attTq_sb = persist.tile([128, 2, Q], BF16, tag="attTq")
            waxT_sb = persist.tile([128, 2, 2, 128], BF16, tag="waxT")
            wc_sb = persist.tile([128, 4, 2, 128], BF16, tag="wc")
            smalls_sb = persist.tile([128, 8], F32, tag="smalls")
            negones_sb = persist.tile([128, 1], BF16, tag="negones")
            ones_sb = persist.tile([128, 128], BF16, tag="ones")
            ident_sb = persist.tile([32, 32], F32, tag="ident")

            bax = [smalls_sb[:, c:c + 1] for c in range(2)]
            swa = [smalls_sb[:, 2 + c:3 + c] for c in range(2)]   # sqrt(wa)
            bac = [smalls_sb[:, 4 + c:5 + c] for c in range(2)]
            swab = [smalls_sb[:, 6 + c:7 + c] for c in range(2)]  # sqrt(wa)*bax

            # --- input DMAs: ALL bulk on the sync ring in consumption
            # order (srcN after attT so attT chunks never wait behind it);
            # scalar ring takes the small tensors + later the wsq round
            # trips ---
            nc.sync.dma_start(out=waxT_sb[:], in_=waxT_d[:])
            nc.scalar.dma_start(out=smalls_sb[:], in_=smalls_d[:])
            nc.sync.dma_start(
                out=attT_sb[0][:, :, 0:1024], in_=attT_d[0, :, :, 0:1024]
            )
            nc.sync.dma_start(
                out=attT_sb[0][:, :, 1024:2048], in_=attT_d[0, :, :, 1024:2048]
            )
            nc.sync.dma_start(out=attTq_sb[:], in_=attTq_d[:])
            for t in range(1, 4):
                for hh in range(2):
                    nc.sync.dma_start(
                        out=attT_sb[t][:, :, hh * 1024:(hh + 1) * 1024],
                        in_=attT_d[t, :, :, hh * 1024:(hh + 1) * 1024],
                    )
            nc.scalar.dma_start(out=srcTq_sb[:], in_=srcTq_d[:])
            nc.scalar.dma_start(out=wc_sb[:], in_=wc_d[:])
            for hh in range(2):
                nc.sync.dma_start(out=srcN_sb[hh][:], in_=srcN_d[hh])

            nc.vector.memset(ones_sb[:], 1.0)
            nc.vector.memset(negones_sb[:], -1.0)
            masks.make_identity(nc, ident_sb[:])

            with tc.tile_pool(name="dr", bufs=1, space="DRAM") as dr, \
                 tc.tile_pool(name="atq", bufs=2) as atq_p, \
                 tc.tile_pool(name="at", bufs=3) as at_p, \
                 tc.tile_pool(name="sq", bufs=3) as sq_p, \
                 tc.tile_pool(name="eT", bufs=9) as eT_p, \
                 tc.tile_pool(name="gN", bufs=2) as gN_p, \
                 tc.tile_pool(name="ct", bufs=2) as ct_p, \
                 tc.tile_pool(name="rcp", bufs=2) as rcp_p, \
                 tc.tile_pool(name="dacc", bufs=2) as dacc_p, \
                 tc.tile_pool(name="psS", bufs=4, space="PSUM") as psS, \
                 tc.tile_pool(name="psW", bufs=1, space="PSUM") as psW, \
                 tc.tile_pool(name="psG", bufs=1, space="PSUM") as psG, \
                 tc.tile_pool(name="psD", bufs=1, space="PSUM") as psD:

                wsq_dram = dr.tile([L], F32, tag="wsq_dram")

                # ============ query transform: aq = a^T[:, own] ============
                def emit_queries():
                    for h in range(NQ):
                        for m in range(2):
                            ps = psS.tile([128, QT], F32, tag="s")
                            for c in range(2):
                                nc.tensor.matmul(
                                    ps[:],
                                    waxT_sb[:, c, m, :],
                                    attTq_sb[:, c, h * QT:(h + 1) * QT],
                                    start=(c == 0), stop=(c == 1),
                                )
                            at_q = atq_p.tile([128, QT], F32, tag="atq")
                            nc.scalar.activation(
                                out=at_q[:], in_=ps[:], func=AF.Tanh
                            )
                            nc.vector.tensor_scalar(
                                aq[:, m, h * QT:(h + 1) * QT], at_q[:],
                                bax[m], swa[m],
                                op0=ALU.add, op1=ALU.mult,
                            )

                # ========== prologue l-tile bodies ==========
                # per l-tile: mma -> tanh(ACT) ->
                #   bT_c = tanh*wa + (wa*bax)   (DVE fused tensor_scalar)
                #   sq_0 = (tanh+bax)^2         (ACT Square with bias)
                #   sq_1 = (tanh+bax)^2         (DVE add, DVE mult)
                #   psW  = -sum_d W_A*sq        (PE rank-1, both chunks)
                mma_state = {}

                def emit_mma(t):
                    pss = []
                    for m in range(2):
                        ps = psS.tile([128, LT], F32, tag="s")
                        for c in range(2):
                            nc.tensor.matmul(
                                ps[:],
                                waxT_sb[:, c, m, :],
                                attT_sb[t // 4][:, c, (t % 4) * LT:(t % 4 + 1) * LT],
                                start=(c == 0), stop=(c == 1),
                            )
                        pss.append(ps)
                    return pss

                def emit_ltile(t):
                    # bT' = sqrt(wa)*(tanh+bax) on DVE (c0) / ACT-Identity
                    # (c1); sq = bT'*bT' in one DVE 16-bit op; the wsq
                    # reduction is then a plain -ones matmul.
                    ps_prev = mma_state.pop(t)
                    at_t = at_p.tile([128, 2, LT], F32, tag="at")
                    for m in range(2):
                        nc.scalar.activation(
                            out=at_t[:, m, :], in_=ps_prev[m][:], func=AF.Tanh,
                        )
                    sq_t = sq_p.tile([128, 2, LT], BF16, tag="sq")
                    cc = t // 4
                    ll = (t % 4) * LT
                    nc.vector.tensor_scalar(
                        bT[cc][:, 0, ll:ll + LT], at_t[:, 0, :],
                        bax[0], swa[0],
                        op0=ALU.add, op1=ALU.mult,
                    )
                    nc.gpsimd.tensor_scalar(
                        bT[cc][:, 1, ll:ll + LT], at_t[:, 1, :],
                        bax[1], swa[1],
                        op0=ALU.add, op1=ALU.mult,
                    )
                    nc.vector.tensor_tensor(
                        out=sq_t[:], in0=bT[cc][:, :, ll:ll + LT],
                        in1=bT[cc][:, :, ll:ll + LT], op=ALU.mult,
                    )
                    ps_w = psW.tile([1, LT], F32, tag="psW")
                    for c in range(2):
                        nc.tensor.matmul(
                            ps_w[:], negones_sb[:], sq_t[:, c, :],
                            start=(c == 0), stop=(c == 1),
                        )
                    nc.vector.tensor_copy(
                        out=negrow[t // 8][0:1, (t % 8) * LT:(t % 8 + 1) * LT],
                        in_=ps_w[:],
                    )
                    if t + 1 < NL:
                        mma_state[t + 1] = emit_mma(t + 1)

                def wsq_dmas(hh, eng):
                    # negrow holds -wsq; round-trip through DRAM to get it
                    # partition-distributed (HWDGE ring, 512B-contiguous
                    # descriptors both ways)
                    eng.dma_start(
                        out=wsq_dram[hh * 4096:(hh + 1) * 4096],
                        in_=negrow[hh][0:1, :],
                    )
                    eng.dma_start(
                        out=wsq64[hh][:],
                        in_=bass.AP(
                            tensor=wsq_dram.tensor,
                            offset=wsq_dram.offset + hh * 4096,
                            ap=[[128, 32], [1, 128]],
                        ),
                    )

                def wsq_transpose(hh):
                    # PE-transpose [32,128] -> bias layout [128, 32]; emitted
                    # a couple of l-tiles after wsq_dmas so the PE never
                    # stalls on the DMA round trip.
                    ps_t = psW.tile([128, 32], F32, tag="psW")
                    nc.tensor.transpose(ps_t[:], wsq64[hh][:], ident_sb[:])
                    nc.vector.tensor_scalar_add(wsqT[hh][:], ps_t[:], -CSHIFT)

                # ========== main-loop q-tile emission (step-driven) ==========
                def make_qtile(h, final_tail=False):
                    aq0 = aq[:, 0, h * QT:(h + 1) * QT]
                    aq1 = aq[:, 1, h * QT:(h + 1) * QT]
                    ps_g = psG.tile([128, 2, QT], F32, tag="psG")
                    den = dacc_p.tile([128, QT], F32, tag="dacc")
                    st = {"ss": [], "es": [], "t": 0}

                    def emit_mm1(t):
                        ps_s = psS.tile([128, QT], F32, tag="s")
                        cc, kk = t // 16, (t % 16) * KT
                        nc.tensor.matmul(
                            ps_s[:], bT[cc][:, 0, kk:kk + KT], aq0,
                            start=True, stop=False,
                        )
                        nc.tensor.matmul(
                            ps_s[:], bT[cc][:, 1, kk:kk + KT], aq1,
                            start=False, stop=True,
                        )
                        return ps_s

                    def emit_exp(t):
                        e_t = eT_p.tile([128, QT], BF16, tag="eT")
                        nc.scalar.activation(
                            out=e_t[:], in_=st["ss"][t][:], func=AF.Exp,
                            bias=wsqT[t // 32][:, t % 32:t % 32 + 1],
                            scale=2.0,
                        )
                        return e_t

                    def prime():
                        st["ss"].append(emit_mm1(0))
                        for t in range(DELAY):
                            st["es"].append(emit_exp(t))
                            st["ss"].append(emit_mm1(t + 1))

                    def step():
                        t = st["t"]
                        st["t"] = t + 1
                        es = st["es"]
                        if t + DELAY < NK:
                            es.append(emit_exp(t + DELAY))
                            st["ss"].append(
                                emit_mm1(t + DELAY + 1) if t + DELAY + 1 < NK
                                else None
                            )
                        for m in range(2):
                            nc.tensor.matmul(
                                ps_g[:, m, :],
                                srcN_sb[t // 32][:, t % 32, m * 128:(m + 1) * 128],
                                es[t][:],
                                start=(t == 0), stop=(t == NK - 1),
                            )
                        if t == 1:
                            nc.vector.tensor_tensor(
                                out=den[:], in0=es[0][:], in1=es[1][:],
                                op=ALU.add,
                            )
                        elif t > 1:
                            nc.vector.tensor_tensor(
                                out=den[:], in0=den[:], in1=es[t][:],
                                op=ALU.add,
                            )

                    def tail():
                        # In the final tail the srcTq halves of both combine
                        # groups are emitted first: they keep the PE busy
                        # while the den broadcast/reciprocal chain resolves.
                        # (Not done for tail0 - holding 2 psS slots that
                        # long would stall qtile1's mm1 ring.)
                        den_bf = rcp_p.tile([128, QT], BF16, tag="denbf")
                        nc.vector.tensor_copy(out=den_bf[:], in_=den[:])
                        ps_cs = []
                        if final_tail:
                            for m in range(2):
                                ps_c = psS.tile([128, QT], F32, tag="s")
                                for c in range(2):
                                    nc.tensor.matmul(
                                        ps_c[:], wc_sb[:, 2 + c, m, :],
                                        srcTq_sb[:, c, h * QT:(h + 1) * QT],
                                        start=(c == 0), stop=False,
                                    )
                                ps_cs.append(ps_c)
                        ps_d = psD.tile([128, QT], F32, tag="psD")
                        nc.tensor.matmul(
                            ps_d[:], ones_sb[:], den_bf[:],
                            start=True, stop=True,
                        )
                        rcp = rcp_p.tile([128, QT], F32, tag="rcp")
                        nc.vector.reciprocal_approx_fast(out=rcp[:], in_=ps_d[:])
                        g_n = gN_p.tile([128, 2, QT], BF16, tag="gN")
                        for m in range(2):
                            nc.vector.tensor_tensor(
                                out=g_n[:, m, :], in0=ps_g[:, m, :],
                                in1=rcp[:], op=ALU.mult,
                            )
                        for m in range(2):
                            if final_tail:
                                ps_c = ps_cs[m]
                                for c in range(2):
                                    nc.tensor.matmul(
                                        ps_c[:], wc_sb[:, c, m, :],
                                        g_n[:, c, :],
                                        start=False, stop=(c == 1),
                                    )
                            else:
                                ps_c = psS.tile([128, QT], F32, tag="s")
                                rhss = [
                                    srcTq_sb[:, 0, h * QT:(h + 1) * QT],
                                    srcTq_sb[:, 1, h * QT:(h + 1) * QT],
                                    g_n[:, 0, :], g_n[:, 1, :],
                                ]
                                cidx = [2, 3, 0, 1]
                                for j in range(4):
                                    nc.tensor.matmul(
                                        ps_c[:], wc_sb[:, cidx[j], m, :],
                                        rhss[j],
                                        start=(j == 0), stop=(j == 3),
                                    )
                            c_t = ct_p.tile([128, QT], F32, tag="ct")
                            nc.scalar.activation(
                                out=c_t[:], in_=ps_c[:], func=AF.Tanh
                            )
                            nc.vector.tensor_scalar_add(c_t[:], c_t[:], bac[m])
                            (nc.sync if m == 0 else nc.scalar).dma_start(
                                out=out_d[m, :, h * QT:(h + 1) * QT],
                                in_=c_t[:],
                            )
                    return prime, step, tail

                # ========== emission schedule ==========
                # phase 1: l-tiles 0-9 (sq1 alternates ACT/DVE); wsq half-0
                #   DMA round trip issued after l7, its PE transpose two
                #   l-tiles later so the PE never waits on it.
                # phase 2: l-tiles 10-15 interleaved with k-tiles 0-23 of
                #   q-tile 0 (4 per l-tile; exp lookahead DELAY=2 stays
                #   inside wsq half 0).
                # phase 3: wsq half-1 round trip covered by k-tiles 24-29,
                #   then the rest of q-tile 0 and all of q-tile 1.
                mma_state[0] = emit_mma(0)
                for t in range(8):
                    emit_ltile(t)
                wsq_dmas(0, nc.scalar)
                emit_queries()
                emit_ltile(8)
                emit_ltile(9)
                wsq_transpose(0)

                prime0, step0, tail0 = make_qtile(0)
                prime0()
                for t in range(10, NL):
                    emit_ltile(t)
                    for _ in range(PRO_STEPS):
                        step0()
                wsq_dmas(1, nc.scalar)
                for _ in range(6):
                    step0()
                wsq_transpose(1)
                for _ in range(NK - PRO_STEPS * 6 - 6):
                    step0()

                # qtile1's mm1 pipeline primes before tail0 so the PE keeps
                # streaming; tail0 must precede qtile1's first mm2 (psG WAR).
                prime1, step1, tail1 = make_qtile(1, final_tail=True)
                prime1()
                tail0()
                for _ in range(NK):
                    step1()
                tail1()

    nc.compile()
    return nc


def _get_nc():
    if "nc" not in _cache:
        _cache["nc"] = _build()
    return _cache["nc"]


def _bf16(x):
    import ml_dtypes

    return np.ascontiguousarray(x, dtype=ml_dtypes.bfloat16)


def _prep_inputs(attended, source, W_A_X, b_A_X, W_A, W_A_combine, b_A_combine):
    f = np.float32
    att = np.asarray(attended, dtype=f)
    src = np.asarray(source, dtype=f)
    attT = _bf16(att.T.reshape(2, 128, 4, 2048).transpose(2, 1, 0, 3))
    srcN = _bf16(src.reshape(2, 32, 128, S).transpose(0, 2, 1, 3))
    waxT = _bf16(
        np.asarray(W_A_X, dtype=f).reshape(2, 128, 2, 128).transpose(1, 0, 2, 3)
    )
    wc = _bf16(
        np.asarray(W_A_combine, dtype=f).reshape(4, 128, 2, 128).transpose(1, 0, 2, 3)
    )
    bax = np.asarray(b_A_X, dtype=f)
    wa = np.asarray(W_A, dtype=f)
    swa = np.sqrt(wa)
    bac = np.asarray(b_A_combine, dtype=f)
    smalls = np.ascontiguousarray(
        np.stack(
            [bax[:128], bax[128:], swa[:128], swa[128:], bac[:128], bac[128:],
             swa[:128] * bax[:128], swa[128:] * bax[128:]],
            axis=1,
        )
    )

    in_maps = []
    for i in range(NCORES):
        sl = slice(i * Q, (i + 1) * Q)
        attTq = _bf16(att[sl].T.reshape(2, 128, Q).transpose(1, 0, 2))
        srcTq = _bf16(src[sl].T.reshape(2, 128, Q).transpose(1, 0, 2))
        in_maps.append({
            "attT": attT, "attTq": attTq, "srcN": srcN, "srcTq": srcTq,
            "waxT": waxT, "wc": wc, "smalls": smalls,
        })
    return in_maps


def _run(in_maps, trace=False):
    from concourse.bass_utils import run_bass_kernel_spmd

    nc = _get_nc()
    res = run_bass_kernel_spmd(nc, in_maps, list(range(NCORES)), trace=trace)
    _cache["last_result"] = res
    out = np.empty((L, S), dtype=np.float32)
    for i in range(NCORES):
        o = res.results[i]["out"]          # [2, 128, Q] = combined^T chunks
        out[i * Q:(i + 1) * Q, :] = np.asarray(o, dtype=np.float32).reshape(S, Q).T
    return out


def kernel(**inputs):
    in_maps = _prep_inputs(**inputs)
    return _run(in_maps, trace=False)
